# revision 1
# baseline (speedup 1.0000x reference)
"""Llama GQA attention (B=2,S=1024,P=1024,E=2048,H=32,KV=8,HD=64) on 8 TRN2 cores.

Sharding: tensor-parallel on the KV-group axis — core c owns KV group c and its
4 query heads. x / cos / sin / mask replicated; Wq/Wk/Wv row-sharded; Wo
column-sharded (partial outputs summed on host); cache sharded on the KV axis.
"""
import os
import sys

for _p in ("/opt/trn_rl_repo",):
    if os.path.isdir(_p) and _p not in sys.path:
        sys.path.insert(0, _p)

import numpy as np
import ml_dtypes

import concourse.bass as bass
import concourse.tile as tile
from concourse import bacc, mybir
from concourse.bass_utils import run_bass_kernel_spmd

B, S, P, E, H, KV, HD = 2, 1024, 1024, 2048, 32, 8, 64
CTX = P + S            # 2048
G = H // KV            # 4 heads per core
T = B * S              # 2048 flattened tokens
N_CORES = 8
OC = G * HD            # 256 output cols per core (q / attn)
BF = mybir.dt.bfloat16
F32 = mybir.dt.float32
nbf = ml_dtypes.bfloat16

NCH = CTX // 128       # 16 key chunks of 128
NTQB = S // 512        # 2 query blocks of 512
NE = E // 128          # 16 embed chunks

_built = {}            # classification key -> compiled Bass module


def _classify(MT):
    """MT = exp(mask).T, shape [CTX, S]. Per (tqb, chunk): 'ones'|'zero'|'mixed'."""
    cls = {}
    for tqb in range(NTQB):
        for c in range(NCH):
            sub = MT[128 * c:128 * (c + 1), 512 * tqb:512 * (tqb + 1)]
            if np.all(sub == 1.0):
                cls[(tqb, c)] = "ones"
            elif np.all(sub == 0.0):
                cls[(tqb, c)] = "zero"
            else:
                cls[(tqb, c)] = "mixed"
    return cls


def _groups(cls, tqb, grp):
    """Key-chunk groups of `grp` for one tq block; a group is skipped only if
    all its chunks are fully masked ('zero')."""
    out = []
    for g in range(NCH // grp):
        chunks = list(range(grp * g, grp * (g + 1)))
        if all(cls[(tqb, c)] == "zero" for c in chunks):
            continue
        out.append(chunks)
    return out


DEFAULT_OPTS = dict(
    grp=2,          # key chunks per score group (psum banks per scores buf)
    sc_bufs=2,      # scores psum bufs
    nonorm=False,   # skip softmax normalization (ablation only — wrong result)
    phase=3,        # ablation: 1=loads only, 2=+projections, 3=full
    direct_odd=False,   # write odd head's normalize output straight to at[64:128]
    no_exp=False, no_mask=False, no_av=False, no_scoremm=False, no_wo=False,
    probs_bufs=3, ilv=True, dma_spread=True,
    wp_bufs=3, attn_bufs=4, ostage_bufs=4,
    evac_engine="any",  # engine for psum->sbuf copies: any|vector|scalar
)


def _build(cls, mixed_list, opts=None):
    o = dict(DEFAULT_OPTS)
    if opts:
        o.update(opts)
    Exp = mybir.ActivationFunctionType.Exp
    midx = {tc: j for j, tc in enumerate(mixed_list)}
    nc = bacc.Bacc(None, target_bir_lowering=False, debug=False)
    _evac = {"any": lambda: nc.any, "vector": lambda: nc.vector,
             "scalar": lambda: nc.scalar}[o["evac_engine"]]
    def evac(out, in_):
        if o["evac_engine"] == "scalar":
            nc.scalar.copy(out, in_)
        else:
            _evac().tensor_copy(out, in_)

    xT = nc.dram_tensor("xT", [E, T], BF, kind="ExternalInput")
    wqT = nc.dram_tensor("wqT", [E, OC], BF, kind="ExternalInput")
    wkT = nc.dram_tensor("wkT", [E, HD], BF, kind="ExternalInput")
    wvT = nc.dram_tensor("wvT", [E, HD], BF, kind="ExternalInput")
    woT = nc.dram_tensor("woT", [OC, E], BF, kind="ExternalInput")
    cosR = nc.dram_tensor("cosR", [B, OC, S], BF, kind="ExternalInput")
    ssinR = nc.dram_tensor("ssinR", [B, OC, S], BF, kind="ExternalInput")
    cacheTk = nc.dram_tensor("cacheTk", [B, HD, P], BF, kind="ExternalInput")
    cacheV = nc.dram_tensor("cacheV", [B, P, HD], BF, kind="ExternalInput")
    if mixed_list:
        maskM = nc.dram_tensor("maskM", [len(mixed_list), 128, 512], BF,
                               kind="ExternalInput")
    out_part = nc.dram_tensor("out_part", [T, E], F32, kind="ExternalOutput")

    with tile.TileContext(nc) as tc:
        with (
            tc.tile_pool(name="persist", bufs=1) as pp,
            tc.tile_pool(name="work", bufs=o["wp_bufs"]) as wp,
            tc.tile_pool(name="probs", bufs=o["probs_bufs"]) as prp,
            tc.tile_pool(name="attn", bufs=o["attn_bufs"]) as ap,
            tc.tile_pool(name="ostage", bufs=o["ostage_bufs"]) as op_,
            tc.tile_pool(name="ps_sc", bufs=o["sc_bufs"], space="PSUM") as ps_sc,
            tc.tile_pool(name="ps_av", bufs=2, space="PSUM") as ps_av,
            tc.tile_pool(name="ps_mm", bufs=2, space="PSUM") as ps_mm,
        ):
            # ---- persistent loads ----
            _dmaeng = [[nc.sync]]
            _dmaeng_late = [nc.sync, nc.gpsimd] if o["dma_spread"] else [nc.sync]
            _dmac = [0]
            def ldma(out, in_):
                e_ = _dmaeng[0][_dmac[0] % len(_dmaeng[0])]
                _dmac[0] += 1
                e_.dma_start(out, in_)
            wk_t = pp.tile([128, NE * HD], BF, tag="wk")
            ldma(wk_t[:].rearrange("p (e h) -> p e h", h=HD),
                 wkT[:].rearrange("(e p) h -> p e h", p=128))
            wq_t = pp.tile([128, NE * OC], BF, tag="wq")
            ldma(wq_t[:].rearrange("p (e h) -> p e h", h=OC),
                 wqT[:].rearrange("(e p) h -> p e h", p=128))
            wv_t = pp.tile([128, NE * HD], BF, tag="wv")
            ldma(wv_t[:].rearrange("p (e h) -> p e h", h=HD),
                 wvT[:].rearrange("(e p) h -> p e h", p=128))
            wq = [wq_t[:, OC * i:OC * (i + 1)] for i in range(NE)]
            wk = [wk_t[:, HD * i:HD * (i + 1)] for i in range(NE)]
            wv = [wv_t[:, HD * i:HD * (i + 1)] for i in range(NE)]
            _xteng = ([nc.gpsimd, nc.scalar] if o["dma_spread"]
                      else [nc.sync])
            xt_t = pp.tile([128, NE * T], BF, tag="xt")
            for j in range(4):
                tsl = slice(512 * j, 512 * (j + 1))
                _xteng[j % len(_xteng)].dma_start(
                    xt_t[:].rearrange("p (e t) -> p e t", t=T)[:, :, tsl],
                    xT[:].rearrange("(e p) t -> p e t", p=128)[:, :, tsl])
            xt = [xt_t[:, T * i:T * (i + 1)] for i in range(NE)]
            wo = []
            for i in range(2):
                t_ = pp.tile([128, E], BF, tag=f"wo{i}")
                ldma(t_[:], woT[128 * i:128 * (i + 1), :])
                wo.append(t_)
            _dmaeng[0] = _dmaeng_late
            cs, sn = [], []
            for b in range(B):
                cb, sb_ = [], []
                for hp in range(2):
                    t_ = pp.tile([128, S], BF, tag=f"cos{b}{hp}")
                    ldma(t_[:], cosR[b, 128 * hp:128 * (hp + 1), :])
                    cb.append(t_)
                    t_ = pp.tile([128, S], BF, tag=f"sin{b}{hp}")
                    ldma(t_[:], ssinR[b, 128 * hp:128 * (hp + 1), :])
                    sb_.append(t_)
                cs.append(cb)
                sn.append(sb_)
            keys, vals = [], []
            for b in range(B):
                kt = pp.tile([128, CTX], BF, tag=f"keys{b}")
                ldma(kt[0:64, 0:P], cacheTk[b])
                ldma(kt[64:128, 0:P], cacheTk[b])
                keys.append(kt)
                vt = pp.tile([128, NCH * 65], BF, tag=f"vals{b}")
                ldma(vt[:, 0:65 * (P // 128)].rearrange(
                         "p (c h) -> p c h", h=65)[:, :, 0:64],
                     cacheV[b].rearrange("(c p) h -> p c h", p=128))
                for k in range(NCH):
                    nc.vector.memset(vt[:, 65 * k + 64:65 * k + 65], 1.0)
                vals.append(vt)
            mt = []
            if mixed_list:
                mt_t = pp.tile([128, len(mixed_list) * 512], BF, tag="maskt")
                ldma(mt_t[:].rearrange("p (j f) -> p j f", f=512),
                     maskM[:].rearrange("j p f -> p j f"))
                mt = [mt_t[:, 512 * j:512 * (j + 1)] for j in range(len(mixed_list))]


            # ---- projections + RoPE ----
            qp = [[None, None] for _ in range(B)]
            for b in range(B if o["phase"] >= 2 else 0):
                tok0 = b * S
                # k projection (transposed) + rope -> keys[b][:, P:]
                for tq2 in range(2):
                    sl = slice(tok0 + 512 * tq2, tok0 + 512 * (tq2 + 1))
                    ps = ps_mm.tile([128, 512], F32, tag="mm512")
                    for e in range(NE):
                        nc.tensor.matmul(ps[0:64, :], wk[e][:, 0:64], xt[e][:, sl],
                                         start=(e == 0), stop=(e == NE - 1))
                    kraw = wp.tile([64, 512], BF, tag="kraw")
                    evac(kraw[:], ps[0:64, :])
                    ksw = wp.tile([64, 512], BF, tag="ksw")
                    nc.gpsimd.tensor_copy(ksw[0:32, :], kraw[32:64, :])
                    nc.gpsimd.tensor_copy(ksw[32:64, :], kraw[0:32, :])
                    csl = slice(512 * tq2, 512 * (tq2 + 1))
                    t1 = wp.tile([64, 512], BF, tag="kt1")
                    nc.vector.tensor_mul(t1[:], kraw[:], cs[b][0][0:64, csl])
                    t2 = wp.tile([64, 512], BF, tag="kt2")
                    nc.vector.tensor_mul(t2[:], ksw[:], sn[b][0][0:64, csl])
                    ksl = slice(P + 512 * tq2, P + 512 * (tq2 + 1))
                    nc.vector.tensor_add(keys[b][0:64, ksl], t1[:], t2[:])
                    nc.sync.dma_start(keys[b][64:128, ksl], keys[b][0:64, ksl])
                # q projection (transposed, head-pair packed) + rope
                for hp in range(2):
                    qt = pp.tile([128, S], BF, tag=f"qp{b}{hp}")
                    qp[b][hp] = qt
                    for tq2 in range(2):
                        sl = slice(tok0 + 512 * tq2, tok0 + 512 * (tq2 + 1))
                        ps = ps_mm.tile([128, 512], F32, tag="mm512")
                        for e in range(NE):
                            nc.tensor.matmul(ps[:], wq[e][:, 128 * hp:128 * (hp + 1)],
                                             xt[e][:, sl],
                                             start=(e == 0), stop=(e == NE - 1))
                        qraw = wp.tile([128, 512], BF, tag="qraw")
                        evac(qraw[:], ps[:])
                        qsw = wp.tile([128, 512], BF, tag="qsw")
                        for u in range(2):
                            nc.gpsimd.tensor_copy(qsw[64 * u:64 * u + 32, :],
                                                  qraw[64 * u + 32:64 * u + 64, :])
                            nc.gpsimd.tensor_copy(qsw[64 * u + 32:64 * u + 64, :],
                                                  qraw[64 * u:64 * u + 32, :])
                        csl = slice(512 * tq2, 512 * (tq2 + 1))
                        t1 = wp.tile([128, 512], BF, tag="qt1")
                        nc.vector.tensor_mul(t1[:], qraw[:], cs[b][hp][:, csl])
                        t2 = wp.tile([128, 512], BF, tag="qt2")
                        nc.vector.tensor_mul(t2[:], qsw[:], sn[b][hp][:, csl])
                        nc.vector.tensor_add(qt[:, csl], t1[:], t2[:])
                # v projection (natural layout) -> vals[b] chunks 8..15
                for tc8 in range(S // 128):
                    ps = ps_mm.tile([128, 512], F32, tag="mm512")
                    for e in range(NE):
                        nc.tensor.matmul(ps[:, 0:64],
                                         xt[e][:, tok0 + 128 * tc8:tok0 + 128 * (tc8 + 1)],
                                         wv[e][:],
                                         start=(e == 0), stop=(e == NE - 1))
                    kk = P // 128 + tc8
                    evac(vals[b][:, 65 * kk:65 * kk + 64], ps[:, 0:64])

            # ---- attention + output projection ----
            for b in range(B if o["phase"] >= 3 else 0):
                for tqb in range(NTQB):
                    at_tiles = []
                    glist = _groups(cls, tqb, o["grp"])
                    for hp in range(2):
                        at = ap.tile([128, 512], BF, tag="attn")
                        at_tiles.append(at)
                        if o["ilv"]:
                            pavs = [ps_av.tile([128, 512], F32, tag="av",
                                               name=f"pav{b}{tqb}{hp}{i_}")
                                    for i_ in range(2)]
                            first = [True, True]
                            for gi, chunks in enumerate(glist):
                                last_g = gi == len(glist) - 1
                                for he in range(2):
                                    qsl = qp[b][hp][64 * he:64 * (he + 1),
                                                    512 * tqb:512 * (tqb + 1)]
                                    psc = ps_sc.tile([128, 512 * o["grp"]], F32,
                                                     tag="scores")
                                    for j, c in enumerate(chunks):
                                        nc.tensor.matmul(
                                            psc[:, 512 * j:512 * (j + 1)],
                                            keys[b][64 * he:64 * (he + 1),
                                                    128 * c:128 * (c + 1)],
                                            qsl, start=True, stop=True)
                                    prb = prp.tile([128, 512 * o["grp"]], BF,
                                                   tag="probs")
                                    nc.scalar.activation(prb[:], psc[:], Exp,
                                                         scale=0.125)
                                    for j, c in enumerate(chunks):
                                        k_ = cls[(tqb, c)]
                                        if k_ == "mixed":
                                            nc.vector.tensor_mul(
                                                prb[:, 512 * j:512 * (j + 1)],
                                                prb[:, 512 * j:512 * (j + 1)],
                                                mt[midx[(tqb, c)]][:])
                                        elif k_ == "zero":
                                            nc.vector.memset(
                                                prb[:, 512 * j:512 * (j + 1)], 0.0)
                                    for j, c in enumerate(chunks):
                                        nc.tensor.matmul(
                                            pavs[he][0:65, :],
                                            vals[b][:, 65 * c:65 * (c + 1)],
                                            prb[:, 512 * j:512 * (j + 1)],
                                            start=first[he],
                                            stop=(last_g and j == len(chunks) - 1))
                                        first[he] = False
                            for he in range(2):
                                rcp = wp.tile([1, 512], F32, tag="rcp")
                                nc.vector.reciprocal(rcp[:], pavs[he][64:65, :])
                                dnb = wp.tile([64, 512], F32, tag="dnb")
                                nc.gpsimd.partition_broadcast(dnb[:], rcp[:])
                                nc.vector.tensor_mul(at[64 * he:64 * (he + 1), :],
                                                     pavs[he][0:64, :], dnb[:])
                            continue
                        for he in range(2):
                            qsl = qp[b][hp][64 * he:64 * (he + 1),
                                            512 * tqb:512 * (tqb + 1)]
                            pav = ps_av.tile([128, 512], F32, tag="av")
                            first = True
                            for gi, chunks in enumerate(glist):
                                psc = ps_sc.tile([128, 512 * o["grp"]], F32, tag="scores")
                                for j, c in enumerate(chunks):
                                    if o["no_scoremm"]:
                                        break
                                    nc.tensor.matmul(
                                        psc[:, 512 * j:512 * (j + 1)],
                                        keys[b][64 * he:64 * (he + 1), 128 * c:128 * (c + 1)],
                                        qsl, start=True, stop=True)
                                prb = prp.tile([128, 512 * o["grp"]], BF, tag="probs")
                                if o["no_exp"]:
                                    nc.vector.tensor_copy(prb[:], psc[:])
                                else:
                                    nc.scalar.activation(prb[:], psc[:], Exp, scale=0.125)
                                for j, c in enumerate(chunks):
                                    k_ = cls[(tqb, c)]
                                    if k_ == "mixed" and not o["no_mask"]:
                                        nc.vector.tensor_mul(
                                            prb[:, 512 * j:512 * (j + 1)],
                                            prb[:, 512 * j:512 * (j + 1)],
                                            mt[midx[(tqb, c)]][:])
                                    elif k_ == "zero":
                                        nc.vector.memset(prb[:, 512 * j:512 * (j + 1)], 0.0)
                                last_g = gi == len(glist) - 1
                                for j, c in enumerate(chunks):
                                    if o["no_av"]:
                                        break
                                    nc.tensor.matmul(
                                        pav[0:65, :],
                                        vals[b][:, 65 * c:65 * (c + 1)],
                                        prb[:, 512 * j:512 * (j + 1)],
                                        start=first,
                                        stop=(last_g and j == len(chunks) - 1))
                                    first = False
                            if o["no_av"]:
                                continue
                            if o["nonorm"]:
                                evac(at[64 * he:64 * (he + 1), :], pav[0:64, :])
                                continue
                            rcp = wp.tile([1, 512], F32, tag="rcp")
                            nc.vector.reciprocal(rcp[:], pav[64:65, :])
                            dnb = wp.tile([64, 512], F32, tag="dnb")
                            nc.gpsimd.partition_broadcast(dnb[:], rcp[:])
                            if he == 0 or o["direct_odd"]:
                                nc.vector.tensor_mul(at[64 * he:64 * (he + 1), :],
                                                     pav[0:64, :], dnb[:])
                            else:
                                tmo = wp.tile([64, 512], BF, tag="tmo")
                                nc.vector.tensor_mul(tmo[:], pav[0:64, :], dnb[:])
                                nc.sync.dma_start(at[64:128, :], tmo[:])
                    # Wo for this (b, tqb)
                    for t4 in range(0 if o["no_wo"] else 4):
                        ost = op_.tile([128, E], F32, tag="ost")
                        for eb in range(4):
                            po = ps_mm.tile([128, 512], F32, tag="mm512")
                            nc.tensor.matmul(po[:], at_tiles[0][:, 128 * t4:128 * (t4 + 1)],
                                             wo[0][:, 512 * eb:512 * (eb + 1)],
                                             start=True, stop=False)
                            nc.tensor.matmul(po[:], at_tiles[1][:, 128 * t4:128 * (t4 + 1)],
                                             wo[1][:, 512 * eb:512 * (eb + 1)],
                                             start=False, stop=True)
                            evac(ost[:, 512 * eb:512 * (eb + 1)], po[:])
                        r0 = b * S + 512 * tqb + 128 * t4
                        ldma(out_part[r0:r0 + 128, :], ost[:])
    nc.compile()
    return nc


def _prep(x, cos, sin, mask, cache_k, cache_v, Wq, Wk, Wv, Wo):
    """Host-side sharding/layout prep. Returns (cls, mixed_list, in_maps)."""
    xf = np.asarray(x, np.float32).reshape(T, E)
    xTn = np.ascontiguousarray(xf.T).astype(nbf)
    Mm = np.exp(np.asarray(mask, np.float32)[0, 0])          # [S, CTX]
    MT = np.ascontiguousarray(Mm.T)                          # [CTX, S]
    cls = _classify(MT)
    mixed_list = sorted(tc for tc, v in cls.items() if v == "mixed")

    sign = np.concatenate([-np.ones(HD // 2, np.float32), np.ones(HD // 2, np.float32)])
    cosn = np.asarray(cos, np.float32)
    sinn = np.asarray(sin, np.float32)
    cosRn = np.stack([np.tile(cosn[b].T, (G, 1)) for b in range(B)]).astype(nbf)
    ssinRn = np.stack([np.tile(sign[:, None] * sinn[b].T, (G, 1)) for b in range(B)]).astype(nbf)

    maskM_np = None
    if mixed_list:
        maskM_np = np.stack([
            MT[128 * c:128 * (c + 1), 512 * tqb:512 * (tqb + 1)]
            for (tqb, c) in mixed_list]).astype(nbf)

    Wqn = np.asarray(Wq, np.float32)
    Wkn = np.asarray(Wk, np.float32)
    Wvn = np.asarray(Wv, np.float32)
    Won = np.asarray(Wo, np.float32)
    ckn = np.asarray(cache_k, np.float32)
    cvn = np.asarray(cache_v, np.float32)

    in_maps = []
    for c in range(N_CORES):
        m = {
            "xT": xTn,
            "wqT": np.ascontiguousarray(Wqn[c * OC:(c + 1) * OC].T).astype(nbf),
            "wkT": np.ascontiguousarray(Wkn[c * HD:(c + 1) * HD].T).astype(nbf),
            "wvT": np.ascontiguousarray(Wvn[c * HD:(c + 1) * HD].T).astype(nbf),
            "woT": np.ascontiguousarray(Won[:, c * OC:(c + 1) * OC].T).astype(nbf),
            "cosR": cosRn,
            "ssinR": ssinRn,
            "cacheTk": np.ascontiguousarray(ckn[:, c, :P].transpose(0, 2, 1)).astype(nbf),
            "cacheV": np.ascontiguousarray(cvn[:, c, :P]).astype(nbf),
        }
        if maskM_np is not None:
            m["maskM"] = maskM_np
        in_maps.append(m)
    return cls, mixed_list, in_maps


def kernel(x, cos, sin, mask, cache_k, cache_v, Wq, Wk, Wv, Wo, start_pos):
    assert int(start_pos) == P, f"kernel hardcodes start_pos={P}, got {start_pos}"
    cls, mixed_list, in_maps = _prep(x, cos, sin, mask, cache_k, cache_v,
                                     Wq, Wk, Wv, Wo)
    key = tuple(sorted(cls.items()))
    if key not in _built:
        _built[key] = _build(cls, mixed_list)
    nc = _built[key]
    res = run_bass_kernel_spmd(nc, in_maps, core_ids=list(range(N_CORES)))
    acc = res.results[0]["out_part"].astype(np.float32).copy()
    for c in range(1, N_CORES):
        acc += res.results[c]["out_part"]
    return acc.reshape(B, S, E)



# revision 2
# speedup vs baseline: 1.0089x; 1.0089x over previous
"""Llama GQA attention (B=2,S=1024,P=1024,E=2048,H=32,KV=8,HD=64) on 8 TRN2 cores.

Sharding: tensor-parallel on the KV-group axis - core c owns KV group c and its
4 query heads (2 head-pairs hp, each with 2 heads he). x replicated; Wq/Wk/Wv
row-sharded; Wo column-sharded (f16 partial outputs summed on host); cache
sharded on the KV axis.

v2 layout: A*V uses probsT as the *stationary* matmul operand and V (with a
ones column for the softmax denominator) as *moving*, so each AV matmul costs
only 65 moving rows. Attention output lands as [q, hd]; normalization is a
per-partition tensor_scalar; the [q, hd]->[hd, q] flip for the Wo projection
goes through the XBAR dma transpose. Score matmuls and exps are trimmed to the
causal window at 128-token granularity; only true diagonal 128x128 blocks get
a mask multiply. Projections are emitted as 256-token pieces and drip-fed
between attention groups to fill tensor-engine bubbles while the activation
engine (exp) paces the attention inner loop.
"""
import os
import sys

for _p in ("/opt/trn_rl_repo",):
    if os.path.isdir(_p) and _p not in sys.path:
        sys.path.insert(0, _p)

import numpy as np
import ml_dtypes

import concourse.bass as bass
import concourse.tile as tile
from concourse import bacc, mybir
from concourse.bass_utils import run_bass_kernel_spmd

B, S, P, E, H, KV, HD = 2, 1024, 1024, 2048, 32, 8, 64
CTX = P + S            # 2048
G = H // KV            # 4 heads per core
T = B * S              # 2048 flattened tokens
N_CORES = 8
OC = G * HD            # 256 output cols per core (q / attn)
BF = mybir.dt.bfloat16
F32 = mybir.dt.float32
F16 = mybir.dt.float16
nbf = ml_dtypes.bfloat16

NCH = CTX // 128       # 16 key chunks of 128
NTQB = S // 512        # 2 query blocks of 512
NE = E // 128          # 16 embed chunks
GRP = 2                # key chunks per score/exp group

_built = {}            # classification key -> compiled Bass module


def _classify(MT):
    """MT = exp(mask).T, shape [CTX, S]. Per (tqb, chunk): 'ones'|'zero'|'mixed'."""
    cls = {}
    for tqb in range(NTQB):
        for c in range(NCH):
            sub = MT[128 * c:128 * (c + 1), 512 * tqb:512 * (tqb + 1)]
            if np.all(sub == 1.0):
                cls[(tqb, c)] = "ones"
            elif np.all(sub == 0.0):
                cls[(tqb, c)] = "zero"
            else:
                cls[(tqb, c)] = "mixed"
    return cls


def _q_lo(tqb, c):
    """First valid local-q column (within the 512 block) for key chunk c."""
    return max(0, min(512, 128 * (c - P // 128) - 512 * tqb))


def _c_last(nt):
    """Last key chunk with any valid key for global q-tile nt (128 q rows)."""
    return min(P // 128 + nt, NCH - 1)


DEFAULT_OPTS = dict(
    interleave=True,
    out_dtype="f16",
    wo_evac=("vector", "vector"),
    proj_evac="vector",
    lag=2,               # groups by which AV trails scores (sw pipelining)
    prb_bufs=4,
    prefix2=False,       # emit q(0,1,tq2=0) inline before attention
    cd=3,                # wo filler cooldown in groups
    hb=False,            # hold 2 wo pieces of (b1,tqb0) for the tail
    qt_ilv=True,         # interleave normalize and dma-transpose per qt
    v_inline=True,       # emit v(0,0..3) inline before attention
    pace=4,              # force one filler piece every N groups (0=off)
)


def _build(cls, mixed_list, opts=None):
    o = dict(DEFAULT_OPTS)
    if opts:
        o.update(opts)
    Exp = mybir.ActivationFunctionType.Exp
    midx = {tc: j for j, tc in enumerate(mixed_list)}
    ODT = {"f16": F16, "f32": F32, "bf16": BF}[o["out_dtype"]]
    nc = bacc.Bacc(None, target_bir_lowering=False, debug=False)

    def eng(name):
        return {"vector": nc.vector, "gpsimd": nc.gpsimd, "scalar": nc.scalar,
                "any": nc.any}[name]

    def evac_proj(out, in_):
        if o["proj_evac"] == "scalar":
            nc.scalar.copy(out, in_)
        else:
            eng(o["proj_evac"]).tensor_copy(out, in_)

    def copy_on(name, out, in_):
        if name == "scalar":
            nc.scalar.copy(out, in_)
        else:
            eng(name).tensor_copy(out, in_)

    xT = nc.dram_tensor("xT", [E, T], BF, kind="ExternalInput")
    wqT = nc.dram_tensor("wqT", [E, OC], BF, kind="ExternalInput")
    wkT = nc.dram_tensor("wkT", [E, HD], BF, kind="ExternalInput")
    wvT = nc.dram_tensor("wvT", [E, HD], BF, kind="ExternalInput")
    woT = nc.dram_tensor("woT", [OC, E], BF, kind="ExternalInput")
    cosP = nc.dram_tensor("cosP", [B, 128, S], BF, kind="ExternalInput")
    sinP = nc.dram_tensor("sinP", [B, 128, S], BF, kind="ExternalInput")
    cacheTk = nc.dram_tensor("cacheTk", [B, HD, P], BF, kind="ExternalInput")
    cacheVr = nc.dram_tensor("cacheVr", [B, 128, (P // 128) * 65], BF,
                             kind="ExternalInput")
    if mixed_list:
        maskD = nc.dram_tensor("maskD", [len(mixed_list), 128, 128], BF,
                               kind="ExternalInput")
    out_part = nc.dram_tensor("out_part", [T, E], ODT, kind="ExternalOutput")

    with tile.TileContext(nc) as tc:
        with (
            tc.tile_pool(name="persist", bufs=1) as pp,
            tc.tile_pool(name="work", bufs=3) as wp,
            tc.tile_pool(name="probs", bufs=o["prb_bufs"]) as prp,
            tc.tile_pool(name="astg", bufs=2) as asp,
            tc.tile_pool(name="att", bufs=8) as ap,
            tc.tile_pool(name="ostage", bufs=3) as op_,
            tc.tile_pool(name="ps_sc", bufs=2, space="PSUM") as ps_sc,
            tc.tile_pool(name="ps_av", bufs=2, space="PSUM") as ps_av,
            tc.tile_pool(name="ps_mm", bufs=2, space="PSUM") as ps_mm,
        ):
            # ---- persistent tiles + loads, in priority order ----
            # All loads go on the scalar (Act) HWDGE queue, issued before the
            # exp stream starts; SP keeps transposes/stores/key-copies only.
            xt_t = pp.tile([128, NE * T], BF, tag="xt")

            def ld_xt(j, e_=None):  # 256-token slice j
                tsl = slice(256 * j, 256 * (j + 1))
                (e_ or nc.scalar).dma_start(
                    xt_t[:].rearrange("p (e t) -> p e t", t=T)[:, :, tsl],
                    xT[:].rearrange("(e p) t -> p e t", p=128)[:, :, tsl])

            wk_t = pp.tile([128, NE * HD], BF, tag="wk")
            nc.sync.dma_start(wk_t[:].rearrange("p (e h) -> p e h", h=HD),
                              wkT[:].rearrange("(e p) h -> p e h", p=128))
            ld_xt(0, nc.sync)
            ld_xt(1, nc.sync)
            wq_t = pp.tile([128, NE * OC], BF, tag="wq")
            nc.sync.dma_start(wq_t[:].rearrange("p (e h) -> p e h", h=OC),
                              wqT[:].rearrange("(e p) h -> p e h", p=128))
            wv_t = pp.tile([128, NE * HD], BF, tag="wv")
            nc.sync.dma_start(wv_t[:].rearrange("p (e h) -> p e h", h=HD),
                              wvT[:].rearrange("(e p) h -> p e h", p=128))
            cs, sn, keys, vals = [], [], [], []
            for b in range(B):
                cs.append(pp.tile([128, S], BF, tag=f"cos{b}", name=f"cos{b}"))
                sn.append(pp.tile([128, S], BF, tag=f"sin{b}", name=f"sin{b}"))
                keys.append(pp.tile([128, CTX], BF, tag=f"keys{b}",
                                    name=f"keys{b}"))
                vals.append(pp.tile([128, NCH * 65], BF, tag=f"vals{b}",
                                    name=f"vals{b}"))

            def ld_bside(b, e_=None):
                e_ = e_ or nc.scalar
                e_.dma_start(cs[b][:], cosP[b])
                e_.dma_start(sn[b][:], sinP[b])
                e_.dma_start(keys[b][0:64, 0:P], cacheTk[b])
                e_.dma_start(keys[b][64:128, 0:P], cacheTk[b])
                e_.dma_start(vals[b][:, 0:65 * (P // 128)], cacheVr[b])
                nc.gpsimd.memset(
                    vals[b][:].rearrange("p (c h) -> p c h", h=65)
                    [:, P // 128:, 64], 1.0)

            ld_bside(0, nc.sync)
            ld_xt(2, nc.sync)
            ld_xt(3, nc.sync)
            mt = []
            if mixed_list:
                mt_t = pp.tile([128, len(mixed_list) * 128], BF, tag="maskt")
                nc.sync.dma_start(
                    mt_t[:].rearrange("p (j f) -> p j f", f=128),
                    maskD[:].rearrange("j p f -> p j f"))
                mt = [mt_t[:, 128 * j:128 * (j + 1)]
                      for j in range(len(mixed_list))]
            for j in range(4, 8):
                ld_xt(j, nc.sync)
            ld_bside(1, nc.sync)
            wo = []
            for i in range(2):
                t_ = pp.tile([128, E], BF, tag=f"wo{i}", name=f"wo{i}")
                nc.sync.dma_start(t_[:], woT[128 * i:128 * (i + 1), :])
                wo.append(t_)

            wq = [wq_t[:, OC * i:OC * (i + 1)] for i in range(NE)]
            wk = [wk_t[:, HD * i:HD * (i + 1)] for i in range(NE)]
            wv = [wv_t[:, HD * i:HD * (i + 1)] for i in range(NE)]
            qp = [[None, None] for _ in range(B)]
            for b in range(B):
                for hp in range(2):
                    qp[b][hp] = pp.tile([128, S], BF, tag=f"qp{b}{hp}",
                                        name=f"qp{b}{hp}")

            def xte(e, sl):
                return xt_t[:].rearrange("p (e t) -> p e t", t=T)[:, e, sl]

            # ---- projection pieces (256-token granularity) ----
            _st = {}

            def k_sub(b, tq2, s2):
                tok0 = b * S + 512 * tq2 + 256 * s2
                sl = slice(tok0, tok0 + 256)
                if s2 == 0:
                    _st[("k", b, tq2)] = ps_mm.tile([128, 512], F32, tag="mm512",
                                                    name=f"kps{b}{tq2}")
                ps = _st[("k", b, tq2)]
                for e in range(NE):
                    nc.tensor.matmul(ps[0:64, 256 * s2:256 * (s2 + 1)],
                                     wk[e][:, 0:64], xte(e, sl),
                                     start=(e == 0 and s2 == 0),
                                     stop=(e == NE - 1), skip_group_check=True)
                if s2 == 1:
                    k_rope(b, tq2, ps)

            def k_rope(b, tq2, ps):
                kraw = wp.tile([64, 512], BF, tag="kraw")
                evac_proj(kraw[:], ps[0:64, :])
                ksw = wp.tile([64, 512], BF, tag="ksw")
                nc.gpsimd.tensor_copy(ksw[0:32, :], kraw[32:64, :])
                nc.vector.tensor_copy(ksw[32:64, :], kraw[0:32, :])
                csl = slice(512 * tq2, 512 * (tq2 + 1))
                t1 = wp.tile([64, 512], BF, tag="kt1")
                nc.vector.tensor_mul(t1[:], kraw[:], cs[b][0:64, csl])
                t2 = wp.tile([64, 512], BF, tag="kt2")
                nc.vector.tensor_mul(t2[:], ksw[:], sn[b][0:64, csl])
                ksl = slice(P + 512 * tq2, P + 512 * (tq2 + 1))
                nc.vector.tensor_add(keys[b][0:64, ksl], t1[:], t2[:])
                nc.gpsimd.tensor_copy(keys[b][64:128, ksl], keys[b][0:64, ksl])

            def q_sub(b, hp, tq2, s2):
                tok0 = b * S + 512 * tq2 + 256 * s2
                sl = slice(tok0, tok0 + 256)
                if s2 == 0:
                    _st[("q", b, hp, tq2)] = ps_mm.tile(
                        [128, 512], F32, tag="mm512", name=f"qps{b}{hp}{tq2}")
                ps = _st[("q", b, hp, tq2)]
                for e in range(NE):
                    nc.tensor.matmul(ps[:, 256 * s2:256 * (s2 + 1)],
                                     wq[e][:, 128 * hp:128 * (hp + 1)],
                                     xte(e, sl),
                                     start=(e == 0 and s2 == 0),
                                     stop=(e == NE - 1), skip_group_check=True)
                if s2 == 1:
                    q_rope(b, hp, tq2, ps)

            def q_rope(b, hp, tq2, ps):
                qraw = wp.tile([128, 512], BF, tag="qraw")
                evac_proj(qraw[:], ps[:])
                qsw = wp.tile([128, 512], BF, tag="qsw")
                for u in range(2):
                    e1, e2 = (nc.gpsimd, nc.vector) if u == 0 else \
                        (nc.vector, nc.gpsimd)
                    e1.tensor_copy(qsw[64 * u:64 * u + 32, :],
                                   qraw[64 * u + 32:64 * u + 64, :])
                    e2.tensor_copy(qsw[64 * u + 32:64 * u + 64, :],
                                   qraw[64 * u:64 * u + 32, :])
                csl = slice(512 * tq2, 512 * (tq2 + 1))
                t1 = wp.tile([128, 512], BF, tag="qt1")
                nc.vector.tensor_mul(t1[:], qraw[:], cs[b][:, csl])
                t2 = wp.tile([128, 512], BF, tag="qt2")
                nc.vector.tensor_mul(t2[:], qsw[:], sn[b][:, csl])
                nc.vector.tensor_add(qp[b][hp][:, csl], t1[:], t2[:])

            def v_piece(b, tc8):
                tok0 = b * S
                ps = ps_mm.tile([128, 512], F32, tag="mm512", name=f"vp{b}{tc8}")
                sl = slice(tok0 + 128 * tc8, tok0 + 128 * (tc8 + 1))
                for e in range(NE):
                    nc.tensor.matmul(ps[:, 0:64], xte(e, sl), wv[e][:],
                                     start=(e == 0), stop=(e == NE - 1))
                kk = P // 128 + tc8
                evac_proj(vals[b][:, 65 * kk:65 * kk + 64], ps[:, 0:64])

            def wo_block(b, tqb, t4, at_t):
                last = (b == 1 and tqb == 1)
                ost = op_.tile([128, E], ODT, tag="ost", name=f"ost{b}{tqb}{t4}")
                for eb in range(4):
                    po = ps_mm.tile([128, 512], F32, tag="mm512",
                                    name=f"po{b}{tqb}{t4}{eb}")
                    nc.tensor.matmul(po[:], at_t[0][:, 128 * t4:128 * (t4 + 1)],
                                     wo[0][:, 512 * eb:512 * (eb + 1)],
                                     start=True, stop=False)
                    nc.tensor.matmul(po[:], at_t[1][:, 128 * t4:128 * (t4 + 1)],
                                     wo[1][:, 512 * eb:512 * (eb + 1)],
                                     start=False, stop=True)
                    if last:
                        e_ = ("vector", "scalar", "scalar", "vector")[eb]
                        if e_ == "scalar":
                            nc.scalar.copy(ost[:, 512 * eb:512 * (eb + 1)],
                                           po[:])
                        else:
                            eng(e_).tensor_copy(
                                ost[:, 512 * eb:512 * (eb + 1)], po[:])
                    else:
                        eng(o["wo_evac"][eb % 2]).tensor_copy(
                            ost[:, 512 * eb:512 * (eb + 1)], po[:])
                r0 = b * S + 512 * tqb + 128 * t4
                (nc.scalar if last else nc.sync).dma_start(
                    out_part[r0:r0 + 128, :], ost[:])

            PIECE_NS = {"k": 1750.0, "q": 1750.0, "v": 500.0, "wo": 1800.0,
                        "k#": 1750.0, "q#": 1750.0}
            cont = [None]   # forced continuation piece (psum-pair safety)

            def run_piece(pc):
                kind = pc[0]
                if kind == "k":
                    _, b, tq2 = pc
                    k_sub(b, tq2, 0)
                    cont[0] = ("k#", b, tq2)
                elif kind == "k#":
                    _, b, tq2 = pc
                    k_sub(b, tq2, 1)
                elif kind == "q":
                    _, b, hp, tq2 = pc
                    q_sub(b, hp, tq2, 0)
                    cont[0] = ("q#", b, hp, tq2)
                elif kind == "q#":
                    _, b, hp, tq2 = pc
                    q_sub(b, hp, tq2, 1)
                elif kind == "v":
                    v_piece(*pc[1:])
                elif kind == "wo":
                    wo_block(*pc[1:])

            filler = []
            holdback = []
            cooldown = []        # (ready_at_group, piece)
            group_ctr = [0]
            deficit = [0.0]

            last_fill = [0]

            def pop_piece():
                if cont[0] is not None:
                    pc, cont[0] = cont[0], None
                    return pc
                return filler.pop(0)

            def fill_budget():
                still = []
                for (rdy, pc) in cooldown:
                    if group_ctr[0] >= rdy:
                        filler.append(pc)
                    else:
                        still.append((rdy, pc))
                cooldown[:] = still
                while cont[0] is not None or filler:
                    nxt = cont[0] if cont[0] is not None else filler[0]
                    paced = (o["pace"] and
                             group_ctr[0] - last_fill[0] >= o["pace"])
                    if deficit[0] >= PIECE_NS[nxt[0]] * 0.5 or paced:
                        pc = pop_piece()
                        deficit[0] -= PIECE_NS[pc[0]]
                        last_fill[0] = group_ctr[0]
                        run_piece(pc)
                    else:
                        break

            def drain():
                filler.extend(pc for (_, pc) in cooldown)
                cooldown.clear()
                while cont[0] is not None or filler:
                    run_piece(pop_piece())

            def ensure(match):
                """Run (now) every queued filler piece matching the prefix."""
                if cont[0] is not None:
                    run_piece(pop_piece())
                keep = []
                for pc in filler:
                    if pc[:len(match)] == match:
                        run_piece(pc)
                        if cont[0] is not None:
                            run_piece(pop_piece())
                    else:
                        keep.append(pc)
                filler[:] = keep

            # ---- attention: flat task stream, AV lags scores by o["lag"] ----
            _att_state = {}

            def emit_sc(tk):
                """Scores + exp + mask for one chunk group; returns prb etc."""
                b, tqb, hp, he, chunks = (tk["b"], tk["tqb"], tk["hp"],
                                          tk["he"], tk["chunks"])
                nt0 = 4 * tqb
                j0 = chunks[0] % GRP
                psc = ps_sc.tile([128, 512 * GRP], F32, tag="scores")
                pe_ns = 0.0
                for c in chunks:
                    j = c % GRP
                    lo = _q_lo(tqb, c)
                    pe_ns += (512 - lo) / 2.4
                    nc.tensor.matmul(
                        psc[:, 512 * j + lo:512 * (j + 1)],
                        keys[b][64 * he:64 * (he + 1), 128 * c:128 * (c + 1)],
                        qp[b][hp][64 * he:64 * (he + 1),
                                  512 * tqb + lo:512 * (tqb + 1)],
                        start=True, stop=True)
                prb = prp.tile([128, 512 * GRP], BF, tag="probs")
                lo_g = 512 * j0 + _q_lo(tqb, chunks[0])
                nc.scalar.activation(prb[:, lo_g:], psc[:, lo_g:], Exp,
                                     scale=0.125)
                for c in chunks:
                    j = c % GRP
                    if cls[(tqb, c)] == "mixed":
                        d = c - P // 128 - nt0
                        assert 0 <= d <= 3, (tqb, c)
                        sl = slice(512 * j + 128 * d, 512 * j + 128 * (d + 1))
                        nc.vector.tensor_mul(prb[:, sl], prb[:, sl],
                                             mt[midx[(tqb, c)]][:])
                tk["prb"] = prb
                tk["act_ns"] = (1024 - lo_g) * 0.833 + 185
                tk["pe_ns"] = pe_ns

            def emit_av(tk):
                b, tqb, hp, he, chunks = (tk["b"], tk["tqb"], tk["hp"],
                                          tk["he"], tk["chunks"])
                nt0 = 4 * tqb
                prb = tk["prb"]
                pkey = ("pav", b, tqb, hp, he)
                if tk["first"]:
                    _att_state[pkey] = ps_av.tile([128, 260], F32, tag="pav",
                                                  name=f"pav{b}{tqb}{hp}{he}")
                    _att_state[("avstart", b, tqb, hp, he)] = True
                pav = _att_state[pkey]
                for c in chunks:
                    j = c % GRP
                    for qt in range(4):
                        nt = nt0 + qt
                        if c > P // 128 + nt:
                            continue
                        tk["pe_ns"] += 65 / 2.4
                        st = _att_state.pop(("avstart", b, tqb, hp, he), False)
                        nc.tensor.matmul(
                            pav[:, 65 * qt:65 * qt + 65],
                            prb[:, 512 * j + 128 * qt:512 * j + 128 * (qt + 1)],
                            vals[b][:, 65 * c:65 * (c + 1)],
                            start=st, stop=(c == _c_last(nt)),
                            skip_group_check=True)
                if tk["last"]:
                    finish_pass(tk, pav)

            def finish_pass(tk, pav):
                b, tqb, hp, he = tk["b"], tk["tqb"], tk["hp"], tk["he"]
                skey = ("astg", b, tqb, hp)
                if he == 0:
                    _att_state[skey] = asp.tile([128, 512], BF, tag="astg",
                                                name=f"astg{b}{tqb}{hp}")
                astg = _att_state[skey]
                rcp = wp.tile([128, 4], F32, tag="rcp")
                nc.vector.reciprocal(
                    rcp[:], pav[:].rearrange("p (a b) -> p a b", b=65)[:, :, 64])
                att = None
                if he == 1:
                    att = ap.tile([128, 512], BF, tag="att",
                                  name=f"att{b}{tqb}{hp}")
                    _att_state[("att", b, tqb, hp)] = att
                tr_eng = (nc.scalar if (b == 1 and tqb == 1 and hp == 1)
                          else nc.sync)
                for qt in range(4):
                    nc.vector.tensor_scalar_mul(
                        astg[:, 128 * qt + 64 * he:128 * qt + 64 * (he + 1)],
                        pav[:, 65 * qt:65 * qt + 64], rcp[:, qt:qt + 1])
                    if he == 1:
                        tr_eng.dma_start_transpose(
                            att[:, 128 * qt:128 * (qt + 1)],
                            astg[:, 128 * qt:128 * (qt + 1)])
                if he == 1 and hp == 1:
                    at_t = [_att_state[("att", b, tqb, 0)],
                            _att_state[("att", b, tqb, 1)]]
                    for t4 in range(4):
                        if o["hb"] and b == 1 and tqb == 0 and t4 >= 2:
                            holdback.append(("wo", b, tqb, t4, at_t))
                        else:
                            cooldown.append((group_ctr[0] + o["cd"],
                                             ("wo", b, tqb, t4, at_t)))
                    if not o["interleave"]:
                        drain()

            pending = []

            def attention(b):
                for tqb in range(NTQB):
                    ensure(("k", b, tqb))
                    for tc8 in range(4 * (tqb + 1)):
                        ensure(("v", b, tc8))
                    groups = []
                    for g in range(NCH // GRP):
                        chunks = [c for c in range(GRP * g, GRP * (g + 1))
                                  if cls[(tqb, c)] != "zero"]
                        if chunks:
                            groups.append(chunks)
                    for hp in range(2):
                        ensure(("q", b, hp, tqb))
                        for he in range(2):
                            for gi, chunks in enumerate(groups):
                                tk = dict(b=b, tqb=tqb, hp=hp, he=he,
                                          chunks=chunks, first=(gi == 0),
                                          last=(gi == len(groups) - 1))
                                group_ctr[0] += 1
                                emit_sc(tk)
                                pending.append(tk)
                                if len(pending) > o["lag"]:
                                    tk2 = pending.pop(0)
                                    emit_av(tk2)
                                    if o["interleave"]:
                                        deficit[0] += max(0.0, tk2["act_ns"]
                                                          - tk2["pe_ns"])
                                        fill_budget()

            # ---- schedule ----
            # b0: K then Q-hp0 inline (attention tqb0/hp0 can then start);
            # everything else becomes filler.
            k_sub(0, 0, 0)
            k_sub(0, 0, 1)
            q_sub(0, 0, 0, 0)
            q_sub(0, 0, 0, 1)
            if o["v_inline"]:
                for tc8 in range(4):
                    v_piece(0, tc8)
            if o["prefix2"]:
                q_sub(0, 1, 0, 0)
                q_sub(0, 1, 0, 1)
            else:
                filler.append(("q", 0, 1, 0))
            if not o["v_inline"]:
                for tc8 in range(4):
                    filler.append(("v", 0, tc8))
            filler.append(("k", 0, 1))
            filler.append(("q", 0, 0, 1))
            filler.append(("q", 0, 1, 1))
            for tc8 in range(4, 8):
                filler.append(("v", 0, tc8))
            for tq2 in range(2):
                filler.append(("k", 1, tq2))
            for tq2 in range(2):
                filler.append(("q", 1, 0, tq2))
            for tc8 in range(4):
                filler.append(("v", 1, tc8))
            for tq2 in range(2):
                filler.append(("q", 1, 1, tq2))
            for tc8 in range(4, 8):
                filler.append(("v", 1, tc8))
            if not o["interleave"]:
                drain()
            attention(0)
            if not o["interleave"]:
                drain()
            attention(1)
            while pending:
                tk2 = pending.pop(0)
                emit_av(tk2)
                if o["interleave"]:
                    deficit[0] += max(0.0, tk2["act_ns"] - tk2["pe_ns"])
                    fill_budget()
            filler[0:0] = holdback
            holdback.clear()
            drain()
    nc.compile()
    return nc


def _prep(x, cos, sin, mask, cache_k, cache_v, Wq, Wk, Wv, Wo):
    """Host-side sharding/layout prep. Returns (cls, mixed_list, in_maps)."""
    xf = np.asarray(x, np.float32).reshape(T, E)
    xTn = np.ascontiguousarray(xf.T).astype(nbf)
    Mm = np.exp(np.asarray(mask, np.float32)[0, 0])          # [S, CTX]
    MT = np.ascontiguousarray(Mm.T)                          # [CTX, S]
    cls = _classify(MT)
    mixed_list = sorted(tc for tc, v in cls.items() if v == "mixed")

    sign = np.concatenate([-np.ones(HD // 2, np.float32),
                           np.ones(HD // 2, np.float32)])
    cosn = np.asarray(cos, np.float32)
    sinn = np.asarray(sin, np.float32)
    cosPn = np.stack([np.tile(cosn[b].T, (2, 1)) for b in range(B)]).astype(nbf)
    sinPn = np.stack([np.tile(sign[:, None] * sinn[b].T, (2, 1))
                      for b in range(B)]).astype(nbf)

    maskD_np = None
    if mixed_list:
        blocks = []
        for (tqb, c) in mixed_list:
            d = c - P // 128 - 4 * tqb
            assert 0 <= d <= 3, (tqb, c)
            q0 = 512 * tqb + 128 * d
            blocks.append(MT[128 * c:128 * (c + 1), q0:q0 + 128])
        maskD_np = np.stack(blocks).astype(nbf)

    Wqn = np.asarray(Wq, np.float32)
    Wkn = np.asarray(Wk, np.float32)
    Wvn = np.asarray(Wv, np.float32)
    Won = np.asarray(Wo, np.float32)
    ckn = np.asarray(cache_k, np.float32)
    cvn = np.asarray(cache_v, np.float32)

    in_maps = []
    for c in range(N_CORES):
        cvr = np.zeros((B, 128, (P // 128), 65), np.float32)
        cvr[:, :, :, 64] = 1.0
        cvr[:, :, :, 0:64] = cvn[:, c, :P].reshape(B, P // 128, 128, HD
                                                   ).transpose(0, 2, 1, 3)
        m = {
            "xT": xTn,
            "wqT": np.ascontiguousarray(Wqn[c * OC:(c + 1) * OC].T).astype(nbf),
            "wkT": np.ascontiguousarray(Wkn[c * HD:(c + 1) * HD].T).astype(nbf),
            "wvT": np.ascontiguousarray(Wvn[c * HD:(c + 1) * HD].T).astype(nbf),
            "woT": np.ascontiguousarray(Won[:, c * OC:(c + 1) * OC].T).astype(nbf),
            "cosP": cosPn,
            "sinP": sinPn,
            "cacheTk": np.ascontiguousarray(
                ckn[:, c, :P].transpose(0, 2, 1)).astype(nbf),
            "cacheVr": np.ascontiguousarray(
                cvr.reshape(B, 128, (P // 128) * 65)).astype(nbf),
        }
        if maskD_np is not None:
            m["maskD"] = maskD_np
        in_maps.append(m)
    return cls, mixed_list, in_maps


def kernel(x, cos, sin, mask, cache_k, cache_v, Wq, Wk, Wv, Wo, start_pos):
    assert int(start_pos) == P, f"kernel hardcodes start_pos={P}, got {start_pos}"
    cls, mixed_list, in_maps = _prep(x, cos, sin, mask, cache_k, cache_v,
                                     Wq, Wk, Wv, Wo)
    key = tuple(sorted(cls.items()))
    if key not in _built:
        _built[key] = _build(cls, mixed_list)
    nc = _built[key]
    res = run_bass_kernel_spmd(nc, in_maps, core_ids=list(range(N_CORES)))
    acc = res.results[0]["out_part"].astype(np.float32).copy()
    for c in range(1, N_CORES):
        acc += res.results[c]["out_part"].astype(np.float32)
    return acc.reshape(B, S, E)


# revision 3
# speedup vs baseline: 1.0284x; 1.0193x over previous
"""Llama GQA attention (B=2,S=1024,P=1024,E=2048,H=32,KV=8,HD=64) on 8 TRN2 cores.

Sharding: tensor-parallel on the KV-group axis - core c owns KV group c and its
4 query heads (2 head-pairs hp, each with 2 heads he). x replicated; Wq/Wk/Wv
row-sharded; Wo column-sharded (f16 partial outputs summed on host); cache
sharded on the KV axis.

v2 layout: A*V uses probsT as the *stationary* matmul operand and V (with a
ones column for the softmax denominator) as *moving*, so each AV matmul costs
only 65 moving rows. Attention output lands as [q, hd]; normalization is a
per-partition tensor_scalar; the [q, hd]->[hd, q] flip for the Wo projection
goes through the XBAR dma transpose. Score matmuls and exps are trimmed to the
causal window at 128-token granularity; only true diagonal 128x128 blocks get
a mask multiply. Projections are emitted as 256-token pieces and drip-fed
between attention groups to fill tensor-engine bubbles while the activation
engine (exp) paces the attention inner loop.
"""
import os
import sys

for _p in ("/opt/trn_rl_repo",):
    if os.path.isdir(_p) and _p not in sys.path:
        sys.path.insert(0, _p)

import numpy as np
import ml_dtypes

import concourse.bass as bass
import concourse.tile as tile
from concourse import bacc, mybir
from concourse.bass_utils import run_bass_kernel_spmd

B, S, P, E, H, KV, HD = 2, 1024, 1024, 2048, 32, 8, 64
CTX = P + S            # 2048
G = H // KV            # 4 heads per core
T = B * S              # 2048 flattened tokens
N_CORES = 8
OC = G * HD            # 256 output cols per core (q / attn)
BF = mybir.dt.bfloat16
F32 = mybir.dt.float32
F16 = mybir.dt.float16
nbf = ml_dtypes.bfloat16

NCH = CTX // 128       # 16 key chunks of 128
NTQB = S // 512        # 2 query blocks of 512
NE = E // 128          # 16 embed chunks
GRP = 2                # key chunks per score/exp group

_built = {}            # classification key -> compiled Bass module


def _classify(MT):
    """MT = exp(mask).T, shape [CTX, S]. Per (tqb, chunk): 'ones'|'zero'|'mixed'."""
    cls = {}
    for tqb in range(NTQB):
        for c in range(NCH):
            sub = MT[128 * c:128 * (c + 1), 512 * tqb:512 * (tqb + 1)]
            if np.all(sub == 1.0):
                cls[(tqb, c)] = "ones"
            elif np.all(sub == 0.0):
                cls[(tqb, c)] = "zero"
            else:
                cls[(tqb, c)] = "mixed"
    return cls


def _q_lo(tqb, c):
    """First valid local-q column (within the 512 block) for key chunk c."""
    return max(0, min(512, 128 * (c - P // 128) - 512 * tqb))


def _c_last(nt):
    """Last key chunk with any valid key for global q-tile nt (128 q rows)."""
    return min(P // 128 + nt, NCH - 1)


DEFAULT_OPTS = dict(
    interleave=True,
    out_dtype="f16",
    wo_evac=("vector", "vector"),
    proj_evac="vector",
    lag=2,               # groups by which AV trails scores (sw pipelining)
    prb_bufs=4,
    prefix2=False,       # emit q(0,1,tq2=0) inline before attention
    cd=3,                # wo filler cooldown in groups
    hb=False,            # hold 2 wo pieces of (b1,tqb0) for the tail
    qt_ilv=True,         # interleave normalize and dma-transpose per qt
    v_inline=True,       # emit v(0,0..3) inline before attention
    pace=4,              # force one filler piece every N groups (0=off)
    burst_ns=2000.0,     # max filler PE-time emitted per group
    def_cap=6000.0,      # deficit accumulation cap
)


def _build(cls, mixed_list, opts=None):
    o = dict(DEFAULT_OPTS)
    if opts:
        o.update(opts)
    Exp = mybir.ActivationFunctionType.Exp
    midx = {tc: j for j, tc in enumerate(mixed_list)}
    ODT = {"f16": F16, "f32": F32, "bf16": BF}[o["out_dtype"]]
    nc = bacc.Bacc(None, target_bir_lowering=False, debug=False)

    def eng(name):
        return {"vector": nc.vector, "gpsimd": nc.gpsimd, "scalar": nc.scalar,
                "any": nc.any}[name]

    def evac_proj(out, in_):
        if o["proj_evac"] == "scalar":
            nc.scalar.copy(out, in_)
        else:
            eng(o["proj_evac"]).tensor_copy(out, in_)

    def copy_on(name, out, in_):
        if name == "scalar":
            nc.scalar.copy(out, in_)
        else:
            eng(name).tensor_copy(out, in_)

    xT = nc.dram_tensor("xT", [E, T], BF, kind="ExternalInput")
    wqT = nc.dram_tensor("wqT", [E, OC], BF, kind="ExternalInput")
    wkT = nc.dram_tensor("wkT", [E, HD], BF, kind="ExternalInput")
    wvT = nc.dram_tensor("wvT", [E, HD], BF, kind="ExternalInput")
    woT = nc.dram_tensor("woT", [OC, E], BF, kind="ExternalInput")
    cosP = nc.dram_tensor("cosP", [B, 128, S], BF, kind="ExternalInput")
    sinP = nc.dram_tensor("sinP", [B, 128, S], BF, kind="ExternalInput")
    cacheTk = nc.dram_tensor("cacheTk", [B, HD, P], BF, kind="ExternalInput")
    cacheVr = nc.dram_tensor("cacheVr", [B, 128, (P // 128) * 65], BF,
                             kind="ExternalInput")
    if mixed_list:
        maskD = nc.dram_tensor("maskD", [len(mixed_list), 128, 128], BF,
                               kind="ExternalInput")
    out_part = nc.dram_tensor("out_part", [T, E], ODT, kind="ExternalOutput")

    with tile.TileContext(nc) as tc:
        with (
            tc.tile_pool(name="persist", bufs=1) as pp,
            tc.tile_pool(name="work", bufs=3) as wp,
            tc.tile_pool(name="probs", bufs=o["prb_bufs"]) as prp,
            tc.tile_pool(name="astg", bufs=2) as asp,
            tc.tile_pool(name="att", bufs=8) as ap,
            tc.tile_pool(name="ostage", bufs=3) as op_,
            tc.tile_pool(name="ps_sc", bufs=2, space="PSUM") as ps_sc,
            tc.tile_pool(name="ps_av", bufs=2, space="PSUM") as ps_av,
            tc.tile_pool(name="ps_mm", bufs=2, space="PSUM") as ps_mm,
        ):
            # ---- persistent tiles + loads, in priority order ----
            # All loads go on the scalar (Act) HWDGE queue, issued before the
            # exp stream starts; SP keeps transposes/stores/key-copies only.
            xt_t = pp.tile([128, NE * T], BF, tag="xt")

            def ld_xt(j, e_=None):  # 256-token slice j
                tsl = slice(256 * j, 256 * (j + 1))
                (e_ or nc.scalar).dma_start(
                    xt_t[:].rearrange("p (e t) -> p e t", t=T)[:, :, tsl],
                    xT[:].rearrange("(e p) t -> p e t", p=128)[:, :, tsl])

            wk_t = pp.tile([128, NE * HD], BF, tag="wk")
            nc.sync.dma_start(wk_t[:].rearrange("p (e h) -> p e h", h=HD),
                              wkT[:].rearrange("(e p) h -> p e h", p=128))
            ld_xt(0, nc.sync)
            ld_xt(1, nc.sync)
            wq_t = pp.tile([128, NE * OC], BF, tag="wq")
            nc.sync.dma_start(wq_t[:].rearrange("p (e h) -> p e h", h=OC),
                              wqT[:].rearrange("(e p) h -> p e h", p=128))
            wv_t = pp.tile([128, NE * HD], BF, tag="wv")
            nc.sync.dma_start(wv_t[:].rearrange("p (e h) -> p e h", h=HD),
                              wvT[:].rearrange("(e p) h -> p e h", p=128))
            cs, sn, keys, vals = [], [], [], []
            for b in range(B):
                cs.append(pp.tile([128, S], BF, tag=f"cos{b}", name=f"cos{b}"))
                sn.append(pp.tile([128, S], BF, tag=f"sin{b}", name=f"sin{b}"))
                keys.append(pp.tile([128, CTX], BF, tag=f"keys{b}",
                                    name=f"keys{b}"))
                vals.append(pp.tile([128, NCH * 65], BF, tag=f"vals{b}",
                                    name=f"vals{b}"))

            def ld_bside(b, e_=None):
                e_ = e_ or nc.scalar
                e_.dma_start(cs[b][:], cosP[b])
                e_.dma_start(sn[b][:], sinP[b])
                e_.dma_start(keys[b][0:64, 0:P], cacheTk[b])
                e_.dma_start(keys[b][64:128, 0:P], cacheTk[b])
                e_.dma_start(vals[b][:, 0:65 * (P // 128)], cacheVr[b])
                nc.gpsimd.memset(
                    vals[b][:].rearrange("p (c h) -> p c h", h=65)
                    [:, P // 128:, 64], 1.0)

            ld_bside(0, nc.sync)
            ld_xt(2, nc.sync)
            ld_xt(3, nc.sync)
            mt = []
            if mixed_list:
                mt_t = pp.tile([128, len(mixed_list) * 128], BF, tag="maskt")
                nc.sync.dma_start(
                    mt_t[:].rearrange("p (j f) -> p j f", f=128),
                    maskD[:].rearrange("j p f -> p j f"))
                mt = [mt_t[:, 128 * j:128 * (j + 1)]
                      for j in range(len(mixed_list))]
            for j in range(4, 8):
                ld_xt(j, nc.sync)
            ld_bside(1, nc.sync)
            wo = []
            for i in range(2):
                t_ = pp.tile([128, E], BF, tag=f"wo{i}", name=f"wo{i}")
                nc.sync.dma_start(t_[:], woT[128 * i:128 * (i + 1), :])
                wo.append(t_)

            wq = [wq_t[:, OC * i:OC * (i + 1)] for i in range(NE)]
            wk = [wk_t[:, HD * i:HD * (i + 1)] for i in range(NE)]
            wv = [wv_t[:, HD * i:HD * (i + 1)] for i in range(NE)]
            qp = [[None, None] for _ in range(B)]
            for b in range(B):
                for hp in range(2):
                    qp[b][hp] = pp.tile([128, S], BF, tag=f"qp{b}{hp}",
                                        name=f"qp{b}{hp}")

            def xte(e, sl):
                return xt_t[:].rearrange("p (e t) -> p e t", t=T)[:, e, sl]

            # ---- projection pieces (256-token granularity) ----
            _st = {}

            def k_sub(b, tq2, s2):
                tok0 = b * S + 512 * tq2 + 256 * s2
                sl = slice(tok0, tok0 + 256)
                if s2 == 0:
                    _st[("k", b, tq2)] = ps_mm.tile([128, 512], F32, tag="mm512",
                                                    name=f"kps{b}{tq2}")
                ps = _st[("k", b, tq2)]
                for e in range(NE):
                    nc.tensor.matmul(ps[0:64, 256 * s2:256 * (s2 + 1)],
                                     wk[e][:, 0:64], xte(e, sl),
                                     start=(e == 0 and s2 == 0),
                                     stop=(e == NE - 1), skip_group_check=True)
                if s2 == 1:
                    k_rope(b, tq2, ps)

            def k_rope(b, tq2, ps):
                kraw = wp.tile([64, 512], BF, tag="kraw")
                evac_proj(kraw[:], ps[0:64, :])
                ksw = wp.tile([64, 512], BF, tag="ksw")
                nc.gpsimd.tensor_copy(ksw[0:32, :], kraw[32:64, :])
                nc.vector.tensor_copy(ksw[32:64, :], kraw[0:32, :])
                csl = slice(512 * tq2, 512 * (tq2 + 1))
                t1 = wp.tile([64, 512], BF, tag="kt1")
                nc.vector.tensor_mul(t1[:], kraw[:], cs[b][0:64, csl])
                t2 = wp.tile([64, 512], BF, tag="kt2")
                nc.vector.tensor_mul(t2[:], ksw[:], sn[b][0:64, csl])
                ksl = slice(P + 512 * tq2, P + 512 * (tq2 + 1))
                nc.vector.tensor_add(keys[b][0:64, ksl], t1[:], t2[:])
                nc.gpsimd.tensor_copy(keys[b][64:128, ksl], keys[b][0:64, ksl])

            def q_sub(b, hp, tq2, s2):
                tok0 = b * S + 512 * tq2 + 256 * s2
                sl = slice(tok0, tok0 + 256)
                if s2 == 0:
                    _st[("q", b, hp, tq2)] = ps_mm.tile(
                        [128, 512], F32, tag="mm512", name=f"qps{b}{hp}{tq2}")
                ps = _st[("q", b, hp, tq2)]
                for e in range(NE):
                    nc.tensor.matmul(ps[:, 256 * s2:256 * (s2 + 1)],
                                     wq[e][:, 128 * hp:128 * (hp + 1)],
                                     xte(e, sl),
                                     start=(e == 0 and s2 == 0),
                                     stop=(e == NE - 1), skip_group_check=True)
                if s2 == 1:
                    q_rope(b, hp, tq2, ps)

            def q_rope(b, hp, tq2, ps):
                qraw = wp.tile([128, 512], BF, tag="qraw")
                evac_proj(qraw[:], ps[:])
                qsw = wp.tile([128, 512], BF, tag="qsw")
                for u in range(2):
                    e1, e2 = (nc.gpsimd, nc.vector) if u == 0 else \
                        (nc.vector, nc.gpsimd)
                    e1.tensor_copy(qsw[64 * u:64 * u + 32, :],
                                   qraw[64 * u + 32:64 * u + 64, :])
                    e2.tensor_copy(qsw[64 * u + 32:64 * u + 64, :],
                                   qraw[64 * u:64 * u + 32, :])
                csl = slice(512 * tq2, 512 * (tq2 + 1))
                t1 = wp.tile([128, 512], BF, tag="qt1")
                nc.vector.tensor_mul(t1[:], qraw[:], cs[b][:, csl])
                t2 = wp.tile([128, 512], BF, tag="qt2")
                nc.vector.tensor_mul(t2[:], qsw[:], sn[b][:, csl])
                nc.vector.tensor_add(qp[b][hp][:, csl], t1[:], t2[:])

            def v_piece(b, tc8):
                tok0 = b * S
                ps = ps_mm.tile([128, 512], F32, tag="mm512", name=f"vp{b}{tc8}")
                sl = slice(tok0 + 128 * tc8, tok0 + 128 * (tc8 + 1))
                for e in range(NE):
                    nc.tensor.matmul(ps[:, 0:64], xte(e, sl), wv[e][:],
                                     start=(e == 0), stop=(e == NE - 1))
                kk = P // 128 + tc8
                evac_proj(vals[b][:, 65 * kk:65 * kk + 64], ps[:, 0:64])

            end_phase = [False]

            def wo_block(b, tqb, t4, at_t):
                last = (b == 1 and tqb == 1) or end_phase[0]
                ost = op_.tile([128, E], ODT, tag="ost", name=f"ost{b}{tqb}{t4}")
                for eb in range(4):
                    po = ps_mm.tile([128, 512], F32, tag="mm512",
                                    name=f"po{b}{tqb}{t4}{eb}")
                    nc.tensor.matmul(po[:], at_t[0][:, 128 * t4:128 * (t4 + 1)],
                                     wo[0][:, 512 * eb:512 * (eb + 1)],
                                     start=True, stop=False)
                    nc.tensor.matmul(po[:], at_t[1][:, 128 * t4:128 * (t4 + 1)],
                                     wo[1][:, 512 * eb:512 * (eb + 1)],
                                     start=False, stop=True)
                    if last:
                        e_ = ("vector", "scalar", "scalar", "vector")[eb]
                        if e_ == "scalar":
                            nc.scalar.copy(ost[:, 512 * eb:512 * (eb + 1)],
                                           po[:])
                        else:
                            eng(e_).tensor_copy(
                                ost[:, 512 * eb:512 * (eb + 1)], po[:])
                    else:
                        eng(o["wo_evac"][eb % 2]).tensor_copy(
                            ost[:, 512 * eb:512 * (eb + 1)], po[:])
                r0 = b * S + 512 * tqb + 128 * t4
                (nc.scalar if last else nc.sync).dma_start(
                    out_part[r0:r0 + 128, :], ost[:])

            PIECE_NS = {"k": 1750.0, "q": 1750.0, "v": 500.0, "wo": 1800.0,
                        "k#": 1750.0, "q#": 1750.0}
            cont = [None]   # forced continuation piece (psum-pair safety)

            def run_piece(pc):
                kind = pc[0]
                if kind == "k":
                    _, b, tq2 = pc
                    k_sub(b, tq2, 0)
                    cont[0] = ("k#", b, tq2)
                elif kind == "k#":
                    _, b, tq2 = pc
                    k_sub(b, tq2, 1)
                elif kind == "q":
                    _, b, hp, tq2 = pc
                    q_sub(b, hp, tq2, 0)
                    cont[0] = ("q#", b, hp, tq2)
                elif kind == "q#":
                    _, b, hp, tq2 = pc
                    q_sub(b, hp, tq2, 1)
                elif kind == "v":
                    v_piece(*pc[1:])
                elif kind == "wo":
                    wo_block(*pc[1:])

            filler = []
            holdback = []
            cooldown = []        # (ready_at_group, piece)
            group_ctr = [0]
            deficit = [0.0]

            last_fill = [0]

            def pop_piece():
                if cont[0] is not None:
                    pc, cont[0] = cont[0], None
                    return pc
                return filler.pop(0)

            def fill_budget():
                still = []
                for (rdy, pc) in cooldown:
                    if group_ctr[0] >= rdy:
                        filler.append(pc)
                    else:
                        still.append((rdy, pc))
                cooldown[:] = still
                burst = 0.0
                while cont[0] is not None or filler:
                    nxt = cont[0] if cont[0] is not None else filler[0]
                    if burst + PIECE_NS[nxt[0]] > o["burst_ns"]:
                        break
                    paced = (o["pace"] and
                             group_ctr[0] - last_fill[0] >= o["pace"])
                    if deficit[0] >= PIECE_NS[nxt[0]] * 0.5 or paced:
                        pc = pop_piece()
                        deficit[0] -= PIECE_NS[pc[0]]
                        burst += PIECE_NS[pc[0]]
                        last_fill[0] = group_ctr[0]
                        run_piece(pc)
                    else:
                        break
                deficit[0] = min(deficit[0], o["def_cap"])

            def drain():
                filler.extend(pc for (_, pc) in cooldown)
                cooldown.clear()
                while cont[0] is not None or filler:
                    run_piece(pop_piece())

            def ensure(match):
                """Run (now) every queued filler piece matching the prefix."""
                if cont[0] is not None:
                    run_piece(pop_piece())
                keep = []
                for pc in filler:
                    if pc[:len(match)] == match:
                        run_piece(pc)
                        if cont[0] is not None:
                            run_piece(pop_piece())
                    else:
                        keep.append(pc)
                filler[:] = keep

            # ---- attention: flat task stream, AV lags scores by o["lag"] ----
            _att_state = {}

            def emit_sc(tk):
                """Scores + exp + mask for one chunk group; returns prb etc."""
                b, tqb, hp, he, chunks = (tk["b"], tk["tqb"], tk["hp"],
                                          tk["he"], tk["chunks"])
                nt0 = 4 * tqb
                j0 = chunks[0] % GRP
                psc = ps_sc.tile([128, 512 * GRP], F32, tag="scores")
                pe_ns = 0.0
                for c in chunks:
                    j = c % GRP
                    lo = _q_lo(tqb, c)
                    pe_ns += (512 - lo) / 2.4
                    nc.tensor.matmul(
                        psc[:, 512 * j + lo:512 * (j + 1)],
                        keys[b][64 * he:64 * (he + 1), 128 * c:128 * (c + 1)],
                        qp[b][hp][64 * he:64 * (he + 1),
                                  512 * tqb + lo:512 * (tqb + 1)],
                        start=True, stop=True)
                prb = prp.tile([128, 512 * GRP], BF, tag="probs")
                lo_g = 512 * j0 + _q_lo(tqb, chunks[0])
                nc.scalar.activation(prb[:, lo_g:], psc[:, lo_g:], Exp,
                                     scale=0.125)
                for c in chunks:
                    j = c % GRP
                    if cls[(tqb, c)] == "mixed":
                        d = c - P // 128 - nt0
                        assert 0 <= d <= 3, (tqb, c)
                        sl = slice(512 * j + 128 * d, 512 * j + 128 * (d + 1))
                        nc.vector.tensor_mul(prb[:, sl], prb[:, sl],
                                             mt[midx[(tqb, c)]][:])
                tk["prb"] = prb
                tk["act_ns"] = (1024 - lo_g) * 0.833 + 185
                tk["pe_ns"] = pe_ns

            def emit_av(tk):
                b, tqb, hp, he, chunks = (tk["b"], tk["tqb"], tk["hp"],
                                          tk["he"], tk["chunks"])
                nt0 = 4 * tqb
                prb = tk["prb"]
                pkey = ("pav", b, tqb, hp, he)
                if tk["first"]:
                    _att_state[pkey] = ps_av.tile([128, 260], F32, tag="pav",
                                                  name=f"pav{b}{tqb}{hp}{he}")
                    _att_state[("avstart", b, tqb, hp, he)] = True
                pav = _att_state[pkey]
                for c in chunks:
                    j = c % GRP
                    for qt in range(4):
                        nt = nt0 + qt
                        if c > P // 128 + nt:
                            continue
                        tk["pe_ns"] += 65 / 2.4
                        st = _att_state.pop(("avstart", b, tqb, hp, he), False)
                        nc.tensor.matmul(
                            pav[:, 65 * qt:65 * qt + 65],
                            prb[:, 512 * j + 128 * qt:512 * j + 128 * (qt + 1)],
                            vals[b][:, 65 * c:65 * (c + 1)],
                            start=st, stop=(c == _c_last(nt)),
                            skip_group_check=True)
                if tk["last"]:
                    finish_pass(tk, pav)

            def finish_pass(tk, pav):
                b, tqb, hp, he = tk["b"], tk["tqb"], tk["hp"], tk["he"]
                skey = ("astg", b, tqb, hp)
                if he == 0:
                    _att_state[skey] = asp.tile([128, 512], BF, tag="astg",
                                                name=f"astg{b}{tqb}{hp}")
                astg = _att_state[skey]
                rcp = wp.tile([128, 4], F32, tag="rcp")
                nc.vector.reciprocal(
                    rcp[:], pav[:].rearrange("p (a b) -> p a b", b=65)[:, :, 64])
                att = None
                if he == 1:
                    att = ap.tile([128, 512], BF, tag="att",
                                  name=f"att{b}{tqb}{hp}")
                    _att_state[("att", b, tqb, hp)] = att
                fin = (b == 1 and tqb == 1 and hp == 1)
                for qt in range(4):
                    nc.vector.tensor_scalar_mul(
                        astg[:, 128 * qt + 64 * he:128 * qt + 64 * (he + 1)],
                        pav[:, 65 * qt:65 * qt + 64], rcp[:, qt:qt + 1])
                    if he == 1:
                        tr_eng = nc.scalar if (fin and qt % 2 == 0) else nc.sync
                        tr_eng.dma_start_transpose(
                            att[:, 128 * qt:128 * (qt + 1)],
                            astg[:, 128 * qt:128 * (qt + 1)])
                if he == 1 and hp == 1:
                    at_t = [_att_state[("att", b, tqb, 0)],
                            _att_state[("att", b, tqb, 1)]]
                    for t4 in range(4):
                        if o["hb"] and b == 1 and tqb == 0 and t4 >= 2:
                            holdback.append(("wo", b, tqb, t4, at_t))
                        else:
                            cooldown.append((group_ctr[0] + o["cd"],
                                             ("wo", b, tqb, t4, at_t)))
                    if not o["interleave"]:
                        drain()

            pending = []

            def attention(b):
                for tqb in range(NTQB):
                    ensure(("k", b, tqb))
                    for tc8 in range(4 * (tqb + 1)):
                        ensure(("v", b, tc8))
                    groups = []
                    for g in range(NCH // GRP):
                        chunks = [c for c in range(GRP * g, GRP * (g + 1))
                                  if cls[(tqb, c)] != "zero"]
                        if chunks:
                            groups.append(chunks)
                    for hp in range(2):
                        ensure(("q", b, hp, tqb))
                        for he in range(2):
                            for gi, chunks in enumerate(groups):
                                tk = dict(b=b, tqb=tqb, hp=hp, he=he,
                                          chunks=chunks, first=(gi == 0),
                                          last=(gi == len(groups) - 1))
                                group_ctr[0] += 1
                                emit_sc(tk)
                                pending.append(tk)
                                if len(pending) > o["lag"]:
                                    tk2 = pending.pop(0)
                                    emit_av(tk2)
                                    if o["interleave"]:
                                        deficit[0] += max(0.0, tk2["act_ns"]
                                                          - tk2["pe_ns"])
                                        fill_budget()

            # ---- schedule ----
            # b0: K then Q-hp0 inline (attention tqb0/hp0 can then start);
            # everything else becomes filler.
            k_sub(0, 0, 0)
            k_sub(0, 0, 1)
            q_sub(0, 0, 0, 0)
            q_sub(0, 0, 0, 1)
            if o["v_inline"]:
                for tc8 in range(4):
                    v_piece(0, tc8)
            if o["prefix2"]:
                q_sub(0, 1, 0, 0)
                q_sub(0, 1, 0, 1)
            else:
                filler.append(("q", 0, 1, 0))
            if not o["v_inline"]:
                for tc8 in range(4):
                    filler.append(("v", 0, tc8))
            filler.append(("k", 0, 1))
            filler.append(("q", 0, 0, 1))
            filler.append(("q", 0, 1, 1))
            for tc8 in range(4, 8):
                filler.append(("v", 0, tc8))
            for tq2 in range(2):
                filler.append(("k", 1, tq2))
            for tq2 in range(2):
                filler.append(("q", 1, 0, tq2))
            for tc8 in range(4):
                filler.append(("v", 1, tc8))
            for tq2 in range(2):
                filler.append(("q", 1, 1, tq2))
            for tc8 in range(4, 8):
                filler.append(("v", 1, tc8))
            if not o["interleave"]:
                drain()
            attention(0)
            if not o["interleave"]:
                drain()
            attention(1)
            while pending:
                emit_av(pending.pop(0))
            filler[0:0] = holdback
            holdback.clear()
            end_phase[0] = True
            drain()
    nc.compile()
    return nc


def _prep(x, cos, sin, mask, cache_k, cache_v, Wq, Wk, Wv, Wo):
    """Host-side sharding/layout prep. Returns (cls, mixed_list, in_maps)."""
    xf = np.asarray(x, np.float32).reshape(T, E)
    xTn = np.ascontiguousarray(xf.T).astype(nbf)
    Mm = np.exp(np.asarray(mask, np.float32)[0, 0])          # [S, CTX]
    MT = np.ascontiguousarray(Mm.T)                          # [CTX, S]
    cls = _classify(MT)
    mixed_list = sorted(tc for tc, v in cls.items() if v == "mixed")

    sign = np.concatenate([-np.ones(HD // 2, np.float32),
                           np.ones(HD // 2, np.float32)])
    cosn = np.asarray(cos, np.float32)
    sinn = np.asarray(sin, np.float32)
    cosPn = np.stack([np.tile(cosn[b].T, (2, 1)) for b in range(B)]).astype(nbf)
    sinPn = np.stack([np.tile(sign[:, None] * sinn[b].T, (2, 1))
                      for b in range(B)]).astype(nbf)

    maskD_np = None
    if mixed_list:
        blocks = []
        for (tqb, c) in mixed_list:
            d = c - P // 128 - 4 * tqb
            assert 0 <= d <= 3, (tqb, c)
            q0 = 512 * tqb + 128 * d
            blocks.append(MT[128 * c:128 * (c + 1), q0:q0 + 128])
        maskD_np = np.stack(blocks).astype(nbf)

    Wqn = np.asarray(Wq, np.float32)
    Wkn = np.asarray(Wk, np.float32)
    Wvn = np.asarray(Wv, np.float32)
    Won = np.asarray(Wo, np.float32)
    ckn = np.asarray(cache_k, np.float32)
    cvn = np.asarray(cache_v, np.float32)

    in_maps = []
    for c in range(N_CORES):
        cvr = np.zeros((B, 128, (P // 128), 65), np.float32)
        cvr[:, :, :, 64] = 1.0
        cvr[:, :, :, 0:64] = cvn[:, c, :P].reshape(B, P // 128, 128, HD
                                                   ).transpose(0, 2, 1, 3)
        m = {
            "xT": xTn,
            "wqT": np.ascontiguousarray(Wqn[c * OC:(c + 1) * OC].T).astype(nbf),
            "wkT": np.ascontiguousarray(Wkn[c * HD:(c + 1) * HD].T).astype(nbf),
            "wvT": np.ascontiguousarray(Wvn[c * HD:(c + 1) * HD].T).astype(nbf),
            "woT": np.ascontiguousarray(Won[:, c * OC:(c + 1) * OC].T).astype(nbf),
            "cosP": cosPn,
            "sinP": sinPn,
            "cacheTk": np.ascontiguousarray(
                ckn[:, c, :P].transpose(0, 2, 1)).astype(nbf),
            "cacheVr": np.ascontiguousarray(
                cvr.reshape(B, 128, (P // 128) * 65)).astype(nbf),
        }
        if maskD_np is not None:
            m["maskD"] = maskD_np
        in_maps.append(m)
    return cls, mixed_list, in_maps


def kernel(x, cos, sin, mask, cache_k, cache_v, Wq, Wk, Wv, Wo, start_pos):
    assert int(start_pos) == P, f"kernel hardcodes start_pos={P}, got {start_pos}"
    cls, mixed_list, in_maps = _prep(x, cos, sin, mask, cache_k, cache_v,
                                     Wq, Wk, Wv, Wo)
    key = tuple(sorted(cls.items()))
    if key not in _built:
        _built[key] = _build(cls, mixed_list)
    nc = _built[key]
    res = run_bass_kernel_spmd(nc, in_maps, core_ids=list(range(N_CORES)))
    acc = res.results[0]["out_part"].astype(np.float32).copy()
    for c in range(1, N_CORES):
        acc += res.results[c]["out_part"].astype(np.float32)
    return acc.reshape(B, S, E)


# revision 5
# speedup vs baseline: 1.0306x; 1.0022x over previous
"""Llama GQA attention (B=2,S=1024,P=1024,E=2048,H=32,KV=8,HD=64) on 8 TRN2 cores.

Sharding: tensor-parallel on the KV-group axis - core c owns KV group c and its
4 query heads (2 head-pairs hp, each with 2 heads he). x replicated; Wq/Wk/Wv
row-sharded; Wo column-sharded (f16 partial outputs summed on host); cache
sharded on the KV axis.

v2 layout: A*V uses probsT as the *stationary* matmul operand and V (with a
ones column for the softmax denominator) as *moving*, so each AV matmul costs
only 65 moving rows. Attention output lands as [q, hd]; normalization is a
per-partition tensor_scalar; the [q, hd]->[hd, q] flip for the Wo projection
goes through the XBAR dma transpose. Score matmuls and exps are trimmed to the
causal window at 128-token granularity; only true diagonal 128x128 blocks get
a mask multiply. Projections are emitted as 256-token pieces and drip-fed
between attention groups to fill tensor-engine bubbles while the activation
engine (exp) paces the attention inner loop.
"""
import os
import sys

for _p in ("/opt/trn_rl_repo",):
    if os.path.isdir(_p) and _p not in sys.path:
        sys.path.insert(0, _p)

import numpy as np
import ml_dtypes

import concourse.bass as bass
import concourse.tile as tile
from concourse import bacc, mybir
from concourse.bass_utils import run_bass_kernel_spmd

B, S, P, E, H, KV, HD = 2, 1024, 1024, 2048, 32, 8, 64
CTX = P + S            # 2048
G = H // KV            # 4 heads per core
T = B * S              # 2048 flattened tokens
N_CORES = 8
OC = G * HD            # 256 output cols per core (q / attn)
BF = mybir.dt.bfloat16
F32 = mybir.dt.float32
F16 = mybir.dt.float16
nbf = ml_dtypes.bfloat16

NCH = CTX // 128       # 16 key chunks of 128
NTQB = S // 512        # 2 query blocks of 512
NE = E // 128          # 16 embed chunks
GRP = 2                # key chunks per score/exp group

_built = {}            # classification key -> compiled Bass module


def _classify(MT):
    """MT = exp(mask).T, shape [CTX, S]. Per (tqb, chunk): 'ones'|'zero'|'mixed'."""
    cls = {}
    for tqb in range(NTQB):
        for c in range(NCH):
            sub = MT[128 * c:128 * (c + 1), 512 * tqb:512 * (tqb + 1)]
            if np.all(sub == 1.0):
                cls[(tqb, c)] = "ones"
            elif np.all(sub == 0.0):
                cls[(tqb, c)] = "zero"
            else:
                cls[(tqb, c)] = "mixed"
    return cls


def _q_lo(tqb, c):
    """First valid local-q column (within the 512 block) for key chunk c."""
    return max(0, min(512, 128 * (c - P // 128) - 512 * tqb))


def _c_last(nt):
    """Last key chunk with any valid key for global q-tile nt (128 q rows)."""
    return min(P // 128 + nt, NCH - 1)


DEFAULT_OPTS = dict(
    interleave=True,
    out_dtype="f16",
    wo_evac=("vector", "vector"),
    proj_evac="vector",
    lag=2,               # groups by which AV trails scores (sw pipelining)
    prb_bufs=4,
    prefix2=False,       # emit q(0,1,tq2=0) inline before attention
    cd=3,                # wo filler cooldown in groups
    hb=False,            # hold 2 wo pieces of (b1,tqb0) for the tail
    qt_ilv=True,         # interleave normalize and dma-transpose per qt
    v_inline=True,       # emit v(0,0..3) inline before attention
    pace=4,              # force one filler piece every N groups (0=off)
    burst_ns=2000.0,     # max filler PE-time emitted per group
    def_cap=6000.0,      # deficit accumulation cap
    wp_bufs=3,
    asp_bufs=2,
    ost_bufs=3,
    b1_act_evac=False,
    lookahead=True,
    nofill_last=False,
    la3=True,
    la_sp=1,
    la_hp=False,
    la_v=True,
)


def _build(cls, mixed_list, opts=None):
    o = dict(DEFAULT_OPTS)
    if opts:
        o.update(opts)
    Exp = mybir.ActivationFunctionType.Exp
    midx = {tc: j for j, tc in enumerate(mixed_list)}
    ODT = {"f16": F16, "f32": F32, "bf16": BF}[o["out_dtype"]]
    nc = bacc.Bacc(None, target_bir_lowering=False, debug=False)

    def eng(name):
        return {"vector": nc.vector, "gpsimd": nc.gpsimd, "scalar": nc.scalar,
                "any": nc.any}[name]

    def evac_proj(out, in_):
        if o["proj_evac"] == "scalar":
            nc.scalar.copy(out, in_)
        else:
            eng(o["proj_evac"]).tensor_copy(out, in_)

    def copy_on(name, out, in_):
        if name == "scalar":
            nc.scalar.copy(out, in_)
        else:
            eng(name).tensor_copy(out, in_)

    xT = nc.dram_tensor("xT", [E, T], BF, kind="ExternalInput")
    wqT = nc.dram_tensor("wqT", [E, OC], BF, kind="ExternalInput")
    wkT = nc.dram_tensor("wkT", [E, HD], BF, kind="ExternalInput")
    wvT = nc.dram_tensor("wvT", [E, HD], BF, kind="ExternalInput")
    woT = nc.dram_tensor("woT", [OC, E], BF, kind="ExternalInput")
    cosP = nc.dram_tensor("cosP", [B, 128, S], BF, kind="ExternalInput")
    sinP = nc.dram_tensor("sinP", [B, 128, S], BF, kind="ExternalInput")
    cacheTk = nc.dram_tensor("cacheTk", [B, HD, P], BF, kind="ExternalInput")
    cacheVr = nc.dram_tensor("cacheVr", [B, 128, (P // 128) * 65], BF,
                             kind="ExternalInput")
    if mixed_list:
        maskD = nc.dram_tensor("maskD", [len(mixed_list), 128, 128], BF,
                               kind="ExternalInput")
    out_part = nc.dram_tensor("out_part", [T, E], ODT, kind="ExternalOutput")

    with tile.TileContext(nc) as tc:
        with (
            tc.tile_pool(name="persist", bufs=1) as pp,
            tc.tile_pool(name="work", bufs=o["wp_bufs"]) as wp,
            tc.tile_pool(name="probs", bufs=o["prb_bufs"]) as prp,
            tc.tile_pool(name="astg", bufs=o["asp_bufs"]) as asp,
            tc.tile_pool(name="att", bufs=8) as ap,
            tc.tile_pool(name="ostage", bufs=o["ost_bufs"]) as op_,
            tc.tile_pool(name="ps_sc", bufs=2, space="PSUM") as ps_sc,
            tc.tile_pool(name="ps_av", bufs=2, space="PSUM") as ps_av,
            tc.tile_pool(name="ps_mm", bufs=2, space="PSUM") as ps_mm,
        ):
            # ---- persistent tiles + loads, in priority order ----
            # Loads go on the SP HWDGE queue; the Act queue is kept clear for
            # the exp stream. A dummy exp up front pulls the activation-table
            # load off the first real exp's critical path.
            warm = pp.tile([1, 2], F32, tag="warm")
            nc.vector.memset(warm[:, 0:1], 0.0)
            nc.scalar.activation(warm[:, 1:2], warm[:, 0:1],
                                 mybir.ActivationFunctionType.Exp, scale=1.0)
            xt_t = pp.tile([128, NE * T], BF, tag="xt")

            def ld_xt(j, e_=None, esplit=False):  # 256-token slice j
                tsl = slice(256 * j, 256 * (j + 1))
                eng_ = e_ or nc.scalar
                if esplit:
                    for lo, hi in ((0, NE // 2), (NE // 2, NE)):
                        eng_.dma_start(
                            xt_t[:].rearrange("p (e t) -> p e t", t=T)
                            [:, lo:hi, tsl],
                            xT[:].rearrange("(e p) t -> p e t", p=128)
                            [:, lo:hi, tsl])
                else:
                    eng_.dma_start(
                        xt_t[:].rearrange("p (e t) -> p e t", t=T)[:, :, tsl],
                        xT[:].rearrange("(e p) t -> p e t", p=128)[:, :, tsl])

            wk_t = pp.tile([128, NE * HD], BF, tag="wk")
            nc.sync.dma_start(wk_t[:].rearrange("p (e h) -> p e h", h=HD),
                              wkT[:].rearrange("(e p) h -> p e h", p=128))
            ld_xt(0, nc.sync)
            ld_xt(1, nc.sync)
            wq_t = pp.tile([128, NE * OC], BF, tag="wq")
            nc.sync.dma_start(wq_t[:].rearrange("p (e h) -> p e h", h=OC),
                              wqT[:].rearrange("(e p) h -> p e h", p=128))
            wv_t = pp.tile([128, NE * HD], BF, tag="wv")
            nc.sync.dma_start(wv_t[:].rearrange("p (e h) -> p e h", h=HD),
                              wvT[:].rearrange("(e p) h -> p e h", p=128))
            cs, sn, keys, vals = [], [], [], []
            for b in range(B):
                cs.append(pp.tile([128, S], BF, tag=f"cos{b}", name=f"cos{b}"))
                sn.append(pp.tile([128, S], BF, tag=f"sin{b}", name=f"sin{b}"))
                keys.append(pp.tile([128, CTX], BF, tag=f"keys{b}",
                                    name=f"keys{b}"))
                vals.append(pp.tile([128, NCH * 65], BF, tag=f"vals{b}",
                                    name=f"vals{b}"))

            def ld_bside(b, e_=None):
                e_ = e_ or nc.scalar
                e_.dma_start(cs[b][:], cosP[b])
                e_.dma_start(sn[b][:], sinP[b])
                e_.dma_start(keys[b][0:64, 0:P], cacheTk[b])
                e_.dma_start(keys[b][64:128, 0:P], cacheTk[b])
                e_.dma_start(vals[b][:, 0:65 * (P // 128)], cacheVr[b])
                nc.gpsimd.memset(
                    vals[b][:].rearrange("p (c h) -> p c h", h=65)
                    [:, P // 128:, 64], 1.0)

            ld_bside(0, nc.sync)
            ld_xt(2, nc.sync)
            ld_xt(3, nc.sync)
            mt = []
            if mixed_list:
                mt_t = pp.tile([128, len(mixed_list) * 128], BF, tag="maskt")
                nc.sync.dma_start(
                    mt_t[:].rearrange("p (j f) -> p j f", f=128),
                    maskD[:].rearrange("j p f -> p j f"))
                mt = [mt_t[:, 128 * j:128 * (j + 1)]
                      for j in range(len(mixed_list))]
            for j in range(4, 8):
                ld_xt(j, nc.sync)
            ld_bside(1, nc.sync)
            wo = []
            for i in range(2):
                t_ = pp.tile([128, E], BF, tag=f"wo{i}", name=f"wo{i}")
                nc.sync.dma_start(t_[:], woT[128 * i:128 * (i + 1), :])
                wo.append(t_)

            wq = [wq_t[:, OC * i:OC * (i + 1)] for i in range(NE)]
            wk = [wk_t[:, HD * i:HD * (i + 1)] for i in range(NE)]
            wv = [wv_t[:, HD * i:HD * (i + 1)] for i in range(NE)]
            qp = [[None, None] for _ in range(B)]
            for b in range(B):
                for hp in range(2):
                    qp[b][hp] = pp.tile([128, S], BF, tag=f"qp{b}{hp}",
                                        name=f"qp{b}{hp}")

            def xte(e, sl):
                return xt_t[:].rearrange("p (e t) -> p e t", t=T)[:, e, sl]

            # ---- projection pieces (256-token granularity) ----
            _st = {}

            def k_sub(b, tq2, s2):
                tok0 = b * S + 512 * tq2 + 256 * s2
                sl = slice(tok0, tok0 + 256)
                if s2 == 0:
                    _st[("k", b, tq2)] = ps_mm.tile([128, 512], F32, tag="mm512",
                                                    name=f"kps{b}{tq2}")
                ps = _st[("k", b, tq2)]
                for e in range(NE):
                    nc.tensor.matmul(ps[0:64, 256 * s2:256 * (s2 + 1)],
                                     wk[e][:, 0:64], xte(e, sl),
                                     start=(e == 0 and s2 == 0),
                                     stop=(e == NE - 1), skip_group_check=True)
                if s2 == 1:
                    k_rope(b, tq2, ps)

            def k_rope(b, tq2, ps):
                kraw = wp.tile([64, 512], BF, tag="kraw")
                evac_proj(kraw[:], ps[0:64, :])
                ksw = wp.tile([64, 512], BF, tag="ksw")
                nc.gpsimd.tensor_copy(ksw[0:32, :], kraw[32:64, :])
                nc.vector.tensor_copy(ksw[32:64, :], kraw[0:32, :])
                csl = slice(512 * tq2, 512 * (tq2 + 1))
                t1 = wp.tile([64, 512], BF, tag="kt1")
                nc.vector.tensor_mul(t1[:], kraw[:], cs[b][0:64, csl])
                t2 = wp.tile([64, 512], BF, tag="kt2")
                nc.vector.tensor_mul(t2[:], ksw[:], sn[b][0:64, csl])
                ksl = slice(P + 512 * tq2, P + 512 * (tq2 + 1))
                nc.vector.tensor_add(keys[b][0:64, ksl], t1[:], t2[:])
                nc.gpsimd.tensor_copy(keys[b][64:128, ksl], keys[b][0:64, ksl])

            def q_sub(b, hp, tq2, s2):
                tok0 = b * S + 512 * tq2 + 256 * s2
                sl = slice(tok0, tok0 + 256)
                if s2 == 0:
                    _st[("q", b, hp, tq2)] = ps_mm.tile(
                        [128, 512], F32, tag="mm512", name=f"qps{b}{hp}{tq2}")
                ps = _st[("q", b, hp, tq2)]
                for e in range(NE):
                    nc.tensor.matmul(ps[:, 256 * s2:256 * (s2 + 1)],
                                     wq[e][:, 128 * hp:128 * (hp + 1)],
                                     xte(e, sl),
                                     start=(e == 0 and s2 == 0),
                                     stop=(e == NE - 1), skip_group_check=True)
                if s2 == 1:
                    q_rope(b, hp, tq2, ps)

            def q_rope(b, hp, tq2, ps):
                qraw = wp.tile([128, 512], BF, tag="qraw")
                evac_proj(qraw[:], ps[:])
                qsw = wp.tile([128, 512], BF, tag="qsw")
                for u in range(2):
                    e1, e2 = (nc.gpsimd, nc.vector) if u == 0 else \
                        (nc.vector, nc.gpsimd)
                    e1.tensor_copy(qsw[64 * u:64 * u + 32, :],
                                   qraw[64 * u + 32:64 * u + 64, :])
                    e2.tensor_copy(qsw[64 * u + 32:64 * u + 64, :],
                                   qraw[64 * u:64 * u + 32, :])
                csl = slice(512 * tq2, 512 * (tq2 + 1))
                t1 = wp.tile([128, 512], BF, tag="qt1")
                nc.vector.tensor_mul(t1[:], qraw[:], cs[b][:, csl])
                t2 = wp.tile([128, 512], BF, tag="qt2")
                nc.vector.tensor_mul(t2[:], qsw[:], sn[b][:, csl])
                nc.vector.tensor_add(qp[b][hp][:, csl], t1[:], t2[:])

            def v_piece(b, tc8):
                tok0 = b * S
                ps = ps_mm.tile([128, 512], F32, tag="mm512", name=f"vp{b}{tc8}")
                sl = slice(tok0 + 128 * tc8, tok0 + 128 * (tc8 + 1))
                for e in range(NE):
                    nc.tensor.matmul(ps[:, 0:64], xte(e, sl), wv[e][:],
                                     start=(e == 0), stop=(e == NE - 1))
                kk = P // 128 + tc8
                evac_proj(vals[b][:, 65 * kk:65 * kk + 64], ps[:, 0:64])

            end_phase = [False]

            def wo_block(b, tqb, t4, at_t):
                last = ((b == 1 and tqb == 1) or end_phase[0]
                        or (o["b1_act_evac"] and b == 1))
                ost = op_.tile([128, E], ODT, tag="ost", name=f"ost{b}{tqb}{t4}")
                for eb in range(4):
                    po = ps_mm.tile([128, 512], F32, tag="mm512",
                                    name=f"po{b}{tqb}{t4}{eb}")
                    nc.tensor.matmul(po[:], at_t[0][:, 128 * t4:128 * (t4 + 1)],
                                     wo[0][:, 512 * eb:512 * (eb + 1)],
                                     start=True, stop=False)
                    nc.tensor.matmul(po[:], at_t[1][:, 128 * t4:128 * (t4 + 1)],
                                     wo[1][:, 512 * eb:512 * (eb + 1)],
                                     start=False, stop=True)
                    if last:
                        e_ = ("vector", "scalar", "scalar", "vector")[eb]
                        if e_ == "scalar":
                            nc.scalar.copy(ost[:, 512 * eb:512 * (eb + 1)],
                                           po[:])
                        else:
                            eng(e_).tensor_copy(
                                ost[:, 512 * eb:512 * (eb + 1)], po[:])
                    else:
                        eng(o["wo_evac"][eb % 2]).tensor_copy(
                            ost[:, 512 * eb:512 * (eb + 1)], po[:])
                r0 = b * S + 512 * tqb + 128 * t4
                (nc.scalar if last else nc.sync).dma_start(
                    out_part[r0:r0 + 128, :], ost[:])

            PIECE_NS = {"k": 1750.0, "q": 1750.0, "v": 500.0, "wo": 1800.0,
                        "k#": 1750.0, "q#": 1750.0}
            cont = [None]   # forced continuation piece (psum-pair safety)

            def run_piece(pc):
                kind = pc[0]
                if kind == "k":
                    _, b, tq2 = pc
                    k_sub(b, tq2, 0)
                    cont[0] = ("k#", b, tq2)
                elif kind == "k#":
                    _, b, tq2 = pc
                    k_sub(b, tq2, 1)
                elif kind == "q":
                    _, b, hp, tq2 = pc
                    q_sub(b, hp, tq2, 0)
                    cont[0] = ("q#", b, hp, tq2)
                elif kind == "q#":
                    _, b, hp, tq2 = pc
                    q_sub(b, hp, tq2, 1)
                elif kind == "v":
                    v_piece(*pc[1:])
                elif kind == "wo":
                    wo_block(*pc[1:])

            filler = []
            holdback = []
            cooldown = []        # (ready_at_group, piece)
            group_ctr = [0]
            deficit = [0.0]

            last_fill = [0]

            def pop_piece():
                if cont[0] is not None:
                    pc, cont[0] = cont[0], None
                    return pc
                return filler.pop(0)

            def fill_budget():
                still = []
                for (rdy, pc) in cooldown:
                    if group_ctr[0] >= rdy:
                        filler.append(pc)
                    else:
                        still.append((rdy, pc))
                cooldown[:] = still
                burst = 0.0
                while cont[0] is not None or filler:
                    nxt = cont[0] if cont[0] is not None else filler[0]
                    if burst + PIECE_NS[nxt[0]] > o["burst_ns"]:
                        break
                    paced = (o["pace"] and
                             group_ctr[0] - last_fill[0] >= o["pace"])
                    if deficit[0] >= PIECE_NS[nxt[0]] * 0.5 or paced:
                        pc = pop_piece()
                        deficit[0] -= PIECE_NS[pc[0]]
                        burst += PIECE_NS[pc[0]]
                        last_fill[0] = group_ctr[0]
                        run_piece(pc)
                    else:
                        break
                deficit[0] = min(deficit[0], o["def_cap"])

            def drain():
                filler.extend(pc for (_, pc) in cooldown)
                cooldown.clear()
                while cont[0] is not None or filler:
                    run_piece(pop_piece())

            def ensure(match):
                """Run (now) every queued filler piece matching the prefix."""
                if cont[0] is not None:
                    run_piece(pop_piece())
                keep = []
                for pc in filler:
                    if pc[:len(match)] == match:
                        run_piece(pc)
                        if cont[0] is not None:
                            run_piece(pop_piece())
                    else:
                        keep.append(pc)
                filler[:] = keep

            # ---- attention: flat task stream, AV lags scores by o["lag"] ----
            _att_state = {}

            def emit_sc(tk):
                """Scores + exp + mask for one chunk group; returns prb etc."""
                b, tqb, hp, he, chunks = (tk["b"], tk["tqb"], tk["hp"],
                                          tk["he"], tk["chunks"])
                nt0 = 4 * tqb
                j0 = chunks[0] % GRP
                psc = ps_sc.tile([128, 512 * GRP], F32, tag="scores")
                pe_ns = 0.0
                for c in chunks:
                    j = c % GRP
                    lo = _q_lo(tqb, c)
                    pe_ns += (512 - lo) / 2.4
                    nc.tensor.matmul(
                        psc[:, 512 * j + lo:512 * (j + 1)],
                        keys[b][64 * he:64 * (he + 1), 128 * c:128 * (c + 1)],
                        qp[b][hp][64 * he:64 * (he + 1),
                                  512 * tqb + lo:512 * (tqb + 1)],
                        start=True, stop=True)
                prb = prp.tile([128, 512 * GRP], BF, tag="probs")
                lo_g = 512 * j0 + _q_lo(tqb, chunks[0])
                nc.scalar.activation(prb[:, lo_g:], psc[:, lo_g:], Exp,
                                     scale=0.125)
                for c in chunks:
                    j = c % GRP
                    if cls[(tqb, c)] == "mixed":
                        d = c - P // 128 - nt0
                        assert 0 <= d <= 3, (tqb, c)
                        sl = slice(512 * j + 128 * d, 512 * j + 128 * (d + 1))
                        nc.vector.tensor_mul(prb[:, sl], prb[:, sl],
                                             mt[midx[(tqb, c)]][:])
                tk["prb"] = prb
                tk["act_ns"] = (1024 - lo_g) * 0.833 + 185
                tk["pe_ns"] = pe_ns

            def emit_av(tk):
                b, tqb, hp, he, chunks = (tk["b"], tk["tqb"], tk["hp"],
                                          tk["he"], tk["chunks"])
                nt0 = 4 * tqb
                prb = tk["prb"]
                pkey = ("pav", b, tqb, hp, he)
                if tk["first"]:
                    _att_state[pkey] = ps_av.tile([128, 260], F32, tag="pav",
                                                  name=f"pav{b}{tqb}{hp}{he}")
                    _att_state[("avstart", b, tqb, hp, he)] = True
                pav = _att_state[pkey]
                for c in chunks:
                    j = c % GRP
                    for qt in range(4):
                        nt = nt0 + qt
                        if c > P // 128 + nt:
                            continue
                        tk["pe_ns"] += 65 / 2.4
                        st = _att_state.pop(("avstart", b, tqb, hp, he), False)
                        nc.tensor.matmul(
                            pav[:, 65 * qt:65 * qt + 65],
                            prb[:, 512 * j + 128 * qt:512 * j + 128 * (qt + 1)],
                            vals[b][:, 65 * c:65 * (c + 1)],
                            start=st, stop=(c == _c_last(nt)),
                            skip_group_check=True)
                if tk["last"]:
                    finish_pass(tk, pav)

            def finish_pass(tk, pav):
                b, tqb, hp, he = tk["b"], tk["tqb"], tk["hp"], tk["he"]
                skey = ("astg", b, tqb, hp)
                if he == 0:
                    _att_state[skey] = asp.tile([128, 512], BF, tag="astg",
                                                name=f"astg{b}{tqb}{hp}")
                astg = _att_state[skey]
                rcp = wp.tile([128, 4], F32, tag="rcp")
                nc.vector.reciprocal(
                    rcp[:], pav[:].rearrange("p (a b) -> p a b", b=65)[:, :, 64])
                att = None
                if he == 1:
                    att = ap.tile([128, 512], BF, tag="att",
                                  name=f"att{b}{tqb}{hp}")
                    _att_state[("att", b, tqb, hp)] = att
                fin = (b == 1 and tqb == 1 and hp == 1)
                for qt in range(4):
                    nc.vector.tensor_scalar_mul(
                        astg[:, 128 * qt + 64 * he:128 * qt + 64 * (he + 1)],
                        pav[:, 65 * qt:65 * qt + 64], rcp[:, qt:qt + 1])
                    if he == 1:
                        tr_eng = nc.scalar if (fin and qt % 2 == 0) else nc.sync
                        tr_eng.dma_start_transpose(
                            att[:, 128 * qt:128 * (qt + 1)],
                            astg[:, 128 * qt:128 * (qt + 1)])
                if he == 1 and hp == 1:
                    at_t = [_att_state[("att", b, tqb, 0)],
                            _att_state[("att", b, tqb, 1)]]
                    for t4 in range(4):
                        if o["hb"] and b == 1 and tqb == 0 and t4 >= 2:
                            holdback.append(("wo", b, tqb, t4, at_t))
                        else:
                            cooldown.append((group_ctr[0] + o["cd"],
                                             ("wo", b, tqb, t4, at_t)))
                    if not o["interleave"]:
                        drain()

            pending = []

            def attention(b):
                for tqb in range(NTQB):
                    ensure(("k", b, tqb))
                    for tc8 in range(4 * (tqb + 1)):
                        ensure(("v", b, tc8))
                    groups = []
                    for g in range(NCH // GRP):
                        chunks = [c for c in range(GRP * g, GRP * (g + 1))
                                  if cls[(tqb, c)] != "zero"]
                        if chunks:
                            groups.append(chunks)
                    lookahead = []
                    if o["lookahead"]:
                        if tqb + 1 < NTQB:
                            lookahead = [("k", b, tqb + 1),
                                         ("q", b, 0, tqb + 1)]
                            if o["la3"]:
                                lookahead.append(("q", b, 1, tqb + 1))
                            if o["la_v"]:
                                lookahead += [("v", b, tc8)
                                              for tc8 in range(4, 8)]
                        elif b + 1 < B:
                            lookahead = [("k", b + 1, 0), ("q", b + 1, 0, 0)]
                            if o["la3"]:
                                lookahead.append(("q", b + 1, 1, 0))
                            if o["la_v"]:
                                lookahead += [("v", b + 1, tc8)
                                              for tc8 in range(4)]
                    ngrp = len(groups)
                    for hp in range(2):
                        ensure(("q", b, hp, tqb))
                        for he in range(2):
                            for gi, chunks in enumerate(groups):
                                tk = dict(b=b, tqb=tqb, hp=hp, he=he,
                                          chunks=chunks, first=(gi == 0),
                                          last=(gi == len(groups) - 1))
                                group_ctr[0] += 1
                                emit_sc(tk)
                                pending.append(tk)
                                if len(pending) > o["lag"]:
                                    tk2 = pending.pop(0)
                                    emit_av(tk2)
                                    if (hp == 1 and he == 1 and lookahead
                                            and gi >= ngrp
                                            - o["la_sp"] * len(lookahead)
                                            and (gi % o["la_sp"] == 0
                                                 or o["la_sp"] == 1)):
                                        ensure(lookahead.pop(0))
                                    if (o["la_hp"] and hp == 0 and he == 1
                                            and gi == ngrp - 1):
                                        ensure(("q", b, 1, tqb))
                                    no_fill = (o["nofill_last"] and b == 1
                                               and tqb == 1)
                                    if o["interleave"] and not no_fill:
                                        deficit[0] += max(0.0, tk2["act_ns"]
                                                          - tk2["pe_ns"])
                                        fill_budget()

            # ---- schedule ----
            # b0: K then Q-hp0 inline (attention tqb0/hp0 can then start);
            # everything else becomes filler.
            k_sub(0, 0, 0)
            k_sub(0, 0, 1)
            q_sub(0, 0, 0, 0)
            q_sub(0, 0, 0, 1)
            if o["v_inline"]:
                for tc8 in range(4):
                    v_piece(0, tc8)
            if o["prefix2"]:
                q_sub(0, 1, 0, 0)
                q_sub(0, 1, 0, 1)
            else:
                filler.append(("q", 0, 1, 0))
            if not o["v_inline"]:
                for tc8 in range(4):
                    filler.append(("v", 0, tc8))
            filler.append(("k", 0, 1))
            filler.append(("q", 0, 0, 1))
            filler.append(("q", 0, 1, 1))
            for tc8 in range(4, 8):
                filler.append(("v", 0, tc8))
            for tq2 in range(2):
                filler.append(("k", 1, tq2))
            for tq2 in range(2):
                filler.append(("q", 1, 0, tq2))
            for tc8 in range(4):
                filler.append(("v", 1, tc8))
            for tq2 in range(2):
                filler.append(("q", 1, 1, tq2))
            for tc8 in range(4, 8):
                filler.append(("v", 1, tc8))
            if not o["interleave"]:
                drain()
            attention(0)
            if not o["interleave"]:
                drain()
            attention(1)
            while pending:
                emit_av(pending.pop(0))
            filler[0:0] = holdback
            holdback.clear()
            end_phase[0] = True
            drain()
    nc.compile()
    return nc


def _prep(x, cos, sin, mask, cache_k, cache_v, Wq, Wk, Wv, Wo):
    """Host-side sharding/layout prep. Returns (cls, mixed_list, in_maps)."""
    xf = np.asarray(x, np.float32).reshape(T, E)
    xTn = np.ascontiguousarray(xf.T).astype(nbf)
    Mm = np.exp(np.asarray(mask, np.float32)[0, 0])          # [S, CTX]
    MT = np.ascontiguousarray(Mm.T)                          # [CTX, S]
    cls = _classify(MT)
    mixed_list = sorted(tc for tc, v in cls.items() if v == "mixed")

    sign = np.concatenate([-np.ones(HD // 2, np.float32),
                           np.ones(HD // 2, np.float32)])
    cosn = np.asarray(cos, np.float32)
    sinn = np.asarray(sin, np.float32)
    cosPn = np.stack([np.tile(cosn[b].T, (2, 1)) for b in range(B)]).astype(nbf)
    sinPn = np.stack([np.tile(sign[:, None] * sinn[b].T, (2, 1))
                      for b in range(B)]).astype(nbf)

    maskD_np = None
    if mixed_list:
        blocks = []
        for (tqb, c) in mixed_list:
            d = c - P // 128 - 4 * tqb
            assert 0 <= d <= 3, (tqb, c)
            q0 = 512 * tqb + 128 * d
            blocks.append(MT[128 * c:128 * (c + 1), q0:q0 + 128])
        maskD_np = np.stack(blocks).astype(nbf)

    Wqn = np.asarray(Wq, np.float32)
    Wkn = np.asarray(Wk, np.float32)
    Wvn = np.asarray(Wv, np.float32)
    Won = np.asarray(Wo, np.float32)
    ckn = np.asarray(cache_k, np.float32)
    cvn = np.asarray(cache_v, np.float32)

    in_maps = []
    for c in range(N_CORES):
        cvr = np.zeros((B, 128, (P // 128), 65), np.float32)
        cvr[:, :, :, 64] = 1.0
        cvr[:, :, :, 0:64] = cvn[:, c, :P].reshape(B, P // 128, 128, HD
                                                   ).transpose(0, 2, 1, 3)
        m = {
            "xT": xTn,
            "wqT": np.ascontiguousarray(Wqn[c * OC:(c + 1) * OC].T).astype(nbf),
            "wkT": np.ascontiguousarray(Wkn[c * HD:(c + 1) * HD].T).astype(nbf),
            "wvT": np.ascontiguousarray(Wvn[c * HD:(c + 1) * HD].T).astype(nbf),
            "woT": np.ascontiguousarray(Won[:, c * OC:(c + 1) * OC].T).astype(nbf),
            "cosP": cosPn,
            "sinP": sinPn,
            "cacheTk": np.ascontiguousarray(
                ckn[:, c, :P].transpose(0, 2, 1)).astype(nbf),
            "cacheVr": np.ascontiguousarray(
                cvr.reshape(B, 128, (P // 128) * 65)).astype(nbf),
        }
        if maskD_np is not None:
            m["maskD"] = maskD_np
        in_maps.append(m)
    return cls, mixed_list, in_maps


def kernel(x, cos, sin, mask, cache_k, cache_v, Wq, Wk, Wv, Wo, start_pos):
    assert int(start_pos) == P, f"kernel hardcodes start_pos={P}, got {start_pos}"
    cls, mixed_list, in_maps = _prep(x, cos, sin, mask, cache_k, cache_v,
                                     Wq, Wk, Wv, Wo)
    key = tuple(sorted(cls.items()))
    if key not in _built:
        _built[key] = _build(cls, mixed_list)
    nc = _built[key]
    res = run_bass_kernel_spmd(nc, in_maps, core_ids=list(range(N_CORES)))
    acc = res.results[0]["out_part"].astype(np.float32).copy()
    for c in range(1, N_CORES):
        acc += res.results[c]["out_part"].astype(np.float32)
    return acc.reshape(B, S, E)
